# revision 25
# baseline (speedup 1.0000x reference)
"""MoE layer (top-2 of 8 experts) for 8 Trainium2 NeuronCores.

Strategy: expert-parallel. Host computes the (tiny) router + top-2 dispatch in
numpy; core e runs expert e's FFN over its dispatched tokens (padded to a fixed
capacity C) with bf16 matmuls (fp32 PSUM accumulation); host combines the two
expert outputs per token.

All device matmuls are [K=128]x[M=128]x[N=512] bf16 (1 cycle/row):
  gate^T/up^T [H, Ct] = gwT/uwT.T @ xt   (contraction over D, 8 k-tiles)
  h = silu(gate) * up                    (SBUF-resident [128, 512] tiles)
  y [Ct, D] = (h.T @ dwT) * p            (contraction over H, 16 k-tiles,
                                          combine-prob scale fused in eviction)

The host pre-packs weights and tokens into SBUF-tile order so every load is a
large DMA with contiguous per-partition lines. v4 schedule: weights are
SBUF-resident (loaded once per invocation at just-in-time program positions),
tokens all hoisted to the start, PE warmup matmuls on a zeroed scratch tile
bridge the initial DMA wait, and redundant per-matmul PE semaphore increments
are stripped post-compile (strip_pe_incs).
"""

import ml_dtypes
import numpy as np

import concourse.bass as bass
import concourse.mybir as mybir
import concourse.tile as tile
from concourse import bacc
from concourse.bass_utils import run_bass_kernel_spmd

E = 8
TOP_K = 2
B, S, D, H = 4, 2048, 1024, 2048
T = B * S
C = 2048          # per-expert token capacity; overflow pairs (seed-0: ~137
                  # of 16384, counts max 2175) fall back to exact host compute
CT = 512          # token tile
P = 128
NCT = C // CT     # 4
KD = D // P       # 8  k-tiles for gate/up
KH = H // P       # 16 k-tiles for down
NH4 = H // 512    # 4  groups of 4 h-blocks
F32 = mybir.dt.float32
F32R = mybir.dt.float32r
# matmul operand dtype: bf16 streams at the same 1 cycle/row as fp32r, but
# enables FWL + LDWEIGHTS pull-ahead (fp32 weight loads serialize inside the
# self-loading matmul) and halves DMA traffic. PSUM accumulation stays fp32.
BF16 = mybir.dt.bfloat16
NP_BF16 = ml_dtypes.bfloat16
MM_DTYPES = {"bf16": (BF16, NP_BF16), "f32r": (F32R, np.float32)}
MM = "bf16"   # default matmul-operand dtype
AF = mybir.ActivationFunctionType


def emit_expert_ffn(tc, xt, gw, uw, dw, pv, y, MMD=None):
    MMD = MMD or MM_DTYPES[MM][0]
    """Emit one expert's FFN.

    DRAM tensors (all pre-packed on host):
      xt [NCT, 128, KD, 512] f32r - tokens, transposed per ct tile
      gw/uw [NH4, 128, KD, 512] f32r - gate/up weights per 4-h-block group
      dw [2, 2, 128, KH//2, 512] f32r - down weights per (dc, kh-half)
      pv [128, C//128] f32 - combine probs (token-partition layout)
      y  [NCT, 2, 128, 4, 512] f32 out - [ct, dc, p, m, 512]
    """
    nc = tc.nc
    # superblocks of up to 2 token tiles sharing one weight pass
    sbs = [list(range(s, min(s + 2, NCT))) for s in range(0, NCT, 2)]

    with (
        tc.tile_pool(name="xpool", bufs=2) as xpool,
        tc.tile_pool(name="wpool", bufs=3) as wpool,
        tc.tile_pool(name="hpool", bufs=36) as hpool,
        tc.tile_pool(name="dpool", bufs=3) as dpool,
        tc.tile_pool(name="tpool", bufs=5) as tpool,
        tc.tile_pool(name="opool", bufs=2) as opool,
        tc.tile_pool(name="ppool", bufs=1) as ppool,
        tc.tile_pool(name="pspool", bufs=8, space="PSUM") as pspool,
    ):
        p_sb = ppool.tile([P, C // P], F32)
        nc.gpsimd.dma_start(p_sb[:, :], pv[:, :])

        for cts in sbs:
            # ---- token tiles: one 2MB DMA per ct ----
            xts = []
            for ct in cts:
                x_t = xpool.tile([P, KD, CT], MMD, name=f"xt_{ct}", tag="xt")
                nc.gpsimd.dma_start(x_t[:, 0:4, :], xt[ct][:, 0:4, :])
                nc.gpsimd.dma_start(x_t[:, 4:8, :], xt[ct][:, 4:8, :])
                xts.append(x_t)
            hs = [
                [
                    hpool.tile([P, CT], MMD, name=f"h_{ct}_{ht}", tag="h")
                    for ht in range(KH)
                ]
                for ct in cts
            ]

            # ---- stage A: gate/up matmuls + silu*mul -> h ----
            first_sb = cts[0] == 0
            for ht4 in range(NH4):
                gt = wpool.tile([P, KD, 512], MMD, name=f"g_{ht4}", tag="w")
                if ht4 == 0 and first_sb:
                    # quarter-granularity on the very first load so the first
                    # matmuls start ~2us earlier out of the cold start
                    for q in range(4):
                        nc.sync.dma_start(
                            gt[:, 2 * q:2 * q + 2, :], gw[ht4][:, 2 * q:2 * q + 2, :]
                        )
                else:
                    nc.sync.dma_start(gt[:, 0:4, :], gw[ht4][:, 0:4, :])
                    nc.sync.dma_start(gt[:, 4:8, :], gw[ht4][:, 4:8, :])
                ut = wpool.tile([P, KD, 512], MMD, name=f"u_{ht4}", tag="w")
                nc.scalar.dma_start(ut[:, 0:4, :], uw[ht4][:, 0:4, :])
                nc.scalar.dma_start(ut[:, 4:8, :], uw[ht4][:, 4:8, :])
                # ct-major, all-gate-then-all-up: gt's last use lands at ~75%
                # of the group so the next group's weight DMA overlaps compute
                for ci in range(len(cts)):
                    tmps = []
                    for sub in range(4):
                        ht = ht4 * 4 + sub
                        pg = pspool.tile([P, CT], F32, name=f"pg_{ht}_{ci}", tag="ps")
                        for kt in range(KD):
                            nc.tensor.matmul(
                                pg[:, :],
                                gt[:, kt, sub * P:(sub + 1) * P],
                                xts[ci][:, kt, :],
                                start=(kt == 0),
                                stop=(kt == KD - 1),
                            )
                        tmp = tpool.tile([P, CT], MMD, name=f"t_{ht}_{ci}", tag="t")
                        nc.scalar.activation(tmp[:, :], pg[:, :], AF.Silu)
                        tmps.append(tmp)
                    for sub in range(4):
                        ht = ht4 * 4 + sub
                        pu = pspool.tile([P, CT], F32, name=f"pu_{ht}_{ci}", tag="ps")
                        for kt in range(KD):
                            nc.tensor.matmul(
                                pu[:, :],
                                ut[:, kt, sub * P:(sub + 1) * P],
                                xts[ci][:, kt, :],
                                start=(kt == 0),
                                stop=(kt == KD - 1),
                            )
                        nc.vector.tensor_mul(
                            hs[ci][ht][:, :], tmps[sub][:, :], pu[:, :]
                        )

            # ---- stage B: down matmuls + prob scale -> y ----
            for dc in range(2):
                pos = {}
                for ci in range(len(cts)):
                    for m in range(CT // P):
                        pos[(ci, m)] = pspool.tile(
                            [P, 512], F32, name=f"po_{dc}_{ci}_{m}", tag="ps"
                        )
                ots = [
                    opool.tile([P, CT // P, 512], F32, name=f"o_{dc}_{ci}", tag="o")
                    for ci in range(len(cts))
                ]
                for hf in range(4):
                    dt_ = dpool.tile([P, KH // 4, 512], MMD, name=f"d_{dc}_{hf}", tag="dw")
                    nc.sync.dma_start(
                        dt_[:, :, :], dw[dc, hf // 2][:, (hf % 2) * 4:(hf % 2) * 4 + 4, :]
                    )
                    for kb in range(KH // 4):
                        kh = hf * (KH // 4) + kb
                        for ci in range(len(cts)):
                            for m in range(CT // P):
                                nc.tensor.matmul(
                                    pos[(ci, m)][:, :],
                                    hs[ci][kh][:, m * P:(m + 1) * P],
                                    dt_[:, kb, :],
                                    start=(kh == 0),
                                    stop=(kh == KH - 1),
                                )
                for ci, ct in enumerate(cts):
                    for m in range(CT // P):
                        j = ct * (CT // P) + m
                        nc.scalar.mul(
                            ots[ci][:, m, :], pos[(ci, m)][:, :], p_sb[:, j:j + 1]
                        )
                        # per-m stores start as soon as each eviction lands,
                        # shortening the kernel-tail drain
                        nc.gpsimd.dma_start(y[ct, dc][:, m, :], ots[ci][:, m, :])


def emit_expert_ffn_v2(tc, xt, gw, uw, dw, pv, y, MMD=None, warm=None):
    """v2: all xt loads hoisted to rep start on the vector DMA queue (first
    ct at quarter granularity for a fast cold start), PE warmup matmuls on a
    zero scratch tile to bridge the head DMA wait (keeps HAM at K=8/8 across
    For_i reps), y stores split across the gpsimd and vector queues."""
    MMD = MMD or MM_DTYPES[MM][0]
    nc = tc.nc
    sbs = [list(range(s, min(s + 2, NCT))) for s in range(0, NCT, 2)]

    with (
        tc.tile_pool(name="xpool", bufs=4) as xpool,
        tc.tile_pool(name="wpool", bufs=3) as wpool,
        tc.tile_pool(name="hpool", bufs=36) as hpool,
        tc.tile_pool(name="dpool", bufs=3) as dpool,
        tc.tile_pool(name="tpool", bufs=5) as tpool,
        tc.tile_pool(name="opool", bufs=2) as opool,
        tc.tile_pool(name="ppool", bufs=1) as ppool,
        tc.tile_pool(name="pspool", bufs=8, space="PSUM") as pspool,
    ):
        p_sb = ppool.tile([P, C // P], F32)
        nc.gpsimd.dma_start(p_sb[:, :], pv[:, :])

        # all token tiles up front on the vector queue
        xts_all = []
        for ct in range(NCT):
            x_t = xpool.tile([P, KD, CT], MMD, name=f"xt_{ct}", tag="xt")
            if ct == 0:
                for q in range(4):
                    nc.gpsimd.dma_start(
                        x_t[:, 2 * q:2 * q + 2, :], xt[ct][:, 2 * q:2 * q + 2, :]
                    )
            else:
                nc.gpsimd.dma_start(x_t[:, 0:4, :], xt[ct][:, 0:4, :])
                nc.gpsimd.dma_start(x_t[:, 4:8, :], xt[ct][:, 4:8, :])
            xts_all.append(x_t)

        # PE warmup: dummy matmuls on the preamble-zeroed scratch tile; no
        # DMA dependencies, so the PE chews these while the first loads land
        if warm is not None:
            wps = pspool.tile([P, 512], F32, name="warm_ps", tag="ps")
            for _ in range(12):
                nc.tensor.matmul(
                    wps[:, :], warm[:, 0:P], warm[:, :], start=True, stop=True
                )

        for cts in sbs:
            xts = [xts_all[ct] for ct in cts]
            hs = [
                [
                    hpool.tile([P, CT], MMD, name=f"h_{ct}_{ht}", tag="h")
                    for ht in range(KH)
                ]
                for ct in cts
            ]

            # ---- stage A: gate/up matmuls + silu*mul -> h ----
            first_sb = cts[0] == 0
            for ht4 in range(NH4):
                gt = wpool.tile([P, KD, 512], MMD, name=f"g_{ht4}", tag="w")
                if ht4 == 0 and first_sb:
                    for q in range(4):
                        nc.sync.dma_start(
                            gt[:, 2 * q:2 * q + 2, :], gw[ht4][:, 2 * q:2 * q + 2, :]
                        )
                else:
                    nc.sync.dma_start(gt[:, 0:4, :], gw[ht4][:, 0:4, :])
                    nc.sync.dma_start(gt[:, 4:8, :], gw[ht4][:, 4:8, :])
                ut = wpool.tile([P, KD, 512], MMD, name=f"u_{ht4}", tag="w")
                nc.scalar.dma_start(ut[:, 0:4, :], uw[ht4][:, 0:4, :])
                nc.scalar.dma_start(ut[:, 4:8, :], uw[ht4][:, 4:8, :])
                for ci in range(len(cts)):
                    tmps = []
                    for sub in range(4):
                        ht = ht4 * 4 + sub
                        pg = pspool.tile([P, CT], F32, name=f"pg_{ht}_{ci}", tag="ps")
                        for kt in range(KD):
                            nc.tensor.matmul(
                                pg[:, :],
                                gt[:, kt, sub * P:(sub + 1) * P],
                                xts[ci][:, kt, :],
                                start=(kt == 0),
                                stop=(kt == KD - 1),
                            )
                        tmp = tpool.tile([P, CT], MMD, name=f"t_{ht}_{ci}", tag="t")
                        nc.scalar.activation(tmp[:, :], pg[:, :], AF.Silu)
                        tmps.append(tmp)
                    for sub in range(4):
                        ht = ht4 * 4 + sub
                        pu = pspool.tile([P, CT], F32, name=f"pu_{ht}_{ci}", tag="ps")
                        for kt in range(KD):
                            nc.tensor.matmul(
                                pu[:, :],
                                ut[:, kt, sub * P:(sub + 1) * P],
                                xts[ci][:, kt, :],
                                start=(kt == 0),
                                stop=(kt == KD - 1),
                            )
                        nc.vector.tensor_mul(
                            hs[ci][ht][:, :], tmps[sub][:, :], pu[:, :]
                        )

            # ---- stage B: down matmuls + prob scale -> y ----
            for dc in range(2):
                pos = {}
                for ci in range(len(cts)):
                    for m in range(CT // P):
                        pos[(ci, m)] = pspool.tile(
                            [P, 512], F32, name=f"po_{dc}_{ci}_{m}", tag="ps"
                        )
                ots = [
                    opool.tile([P, CT // P, 512], F32, name=f"o_{dc}_{ci}", tag="o")
                    for ci in range(len(cts))
                ]
                for hf in range(4):
                    dt_ = dpool.tile([P, KH // 4, 512], MMD, name=f"d_{dc}_{hf}", tag="dw")
                    nc.sync.dma_start(
                        dt_[:, :, :], dw[dc, hf // 2][:, (hf % 2) * 4:(hf % 2) * 4 + 4, :]
                    )
                    for kb in range(KH // 4):
                        kh = hf * (KH // 4) + kb
                        for ci in range(len(cts)):
                            for m in range(CT // P):
                                nc.tensor.matmul(
                                    pos[(ci, m)][:, :],
                                    hs[ci][kh][:, m * P:(m + 1) * P],
                                    dt_[:, kb, :],
                                    start=(kh == 0),
                                    stop=(kh == KH - 1),
                                )
                for ci, ct in enumerate(cts):
                    for m in range(CT // P):
                        j = ct * (CT // P) + m
                        nc.scalar.mul(
                            ots[ci][:, m, :], pos[(ci, m)][:, :], p_sb[:, j:j + 1]
                        )
                        q = nc.gpsimd if m % 2 == 0 else nc.scalar
                        q.dma_start(y[ct, dc][:, m, :], ots[ci][:, m, :])


def emit_expert_ffn_v3(tc, xt, gw, uw, dw, pv, y, MMD=None, warm=None):
    """v3: fully SBUF-resident weights/tokens (bf16 fits: ~186KB/partition).
    All input DMAs issue up front, ordered by first-use time across the three
    queues, so no matmul ever waits on a mid-rep load; the second superblock
    runs with zero input DMAs. Warmup matmuls bridge the initial load."""
    MMD = MMD or MM_DTYPES[MM][0]
    nc = tc.nc
    sbs = [list(range(s, min(s + 2, NCT))) for s in range(0, NCT, 2)]

    with (
        tc.tile_pool(name="xpool", bufs=NCT) as xpool,
        tc.tile_pool(name="gpool", bufs=NH4) as gpool,
        tc.tile_pool(name="upool", bufs=NH4) as upool,
        tc.tile_pool(name="dwpool", bufs=8) as dwpool,
        tc.tile_pool(name="hpool", bufs=36) as hpool,
        tc.tile_pool(name="tpool", bufs=5) as tpool,
        tc.tile_pool(name="opool", bufs=2) as opool,
        tc.tile_pool(name="ppool", bufs=1) as ppool,
        tc.tile_pool(name="pspool", bufs=8, space="PSUM") as pspool,
    ):
        p_sb = ppool.tile([P, C // P], F32)
        nc.gpsimd.dma_start(p_sb[:, :], pv[:, :])

        # ---- all input loads up front, ordered by first use ----
        # gpsimd queue: tokens (ct0 at quarter granularity)
        xts_all = []
        for ct in range(NCT):
            x_t = xpool.tile([P, KD, CT], MMD, name=f"xt_{ct}", tag="xt")
            if ct == 0:
                for q in range(4):
                    nc.gpsimd.dma_start(
                        x_t[:, 2 * q:2 * q + 2, :], xt[ct][:, 2 * q:2 * q + 2, :]
                    )
            else:
                nc.gpsimd.dma_start(x_t[:, 0:4, :], xt[ct][:, 0:4, :])
                nc.gpsimd.dma_start(x_t[:, 4:8, :], xt[ct][:, 4:8, :])
            xts_all.append(x_t)
        # sync queue: gate weights then down dc1; scalar: up weights then dc0
        gts, uts = [], []
        for g in range(NH4):
            gt = gpool.tile([P, KD, 512], MMD, name=f"g_{g}", tag="gw")
            if g == 0:
                for q in range(4):
                    nc.sync.dma_start(
                        gt[:, 2 * q:2 * q + 2, :], gw[g][:, 2 * q:2 * q + 2, :]
                    )
            else:
                nc.sync.dma_start(gt[:, 0:4, :], gw[g][:, 0:4, :])
                nc.sync.dma_start(gt[:, 4:8, :], gw[g][:, 4:8, :])
            gts.append(gt)
        for g in range(NH4):
            ut = upool.tile([P, KD, 512], MMD, name=f"u_{g}", tag="uw")
            nc.scalar.dma_start(ut[:, 0:4, :], uw[g][:, 0:4, :])
            nc.scalar.dma_start(ut[:, 4:8, :], uw[g][:, 4:8, :])
            uts.append(ut)
        dts = {}
        for dc in range(2):
            for hf in range(4):
                dt_ = dwpool.tile(
                    [P, KH // 4, 512], MMD, name=f"d_{dc}_{hf}", tag="dw"
                )
                q = nc.scalar if dc == 0 else nc.sync
                q.dma_start(
                    dt_[:, :, :],
                    dw[dc, hf // 2][:, (hf % 2) * 4:(hf % 2) * 4 + 4, :],
                )
                dts[(dc, hf)] = dt_

        # PE warmup on the preamble-zeroed scratch: no DMA deps
        if warm is not None:
            wps = pspool.tile([P, 512], F32, name="warm_ps", tag="ps")
            for _ in range(12):
                nc.tensor.matmul(
                    wps[:, :], warm[:, 0:P], warm[:, :], start=True, stop=True
                )

        for cts in sbs:
            xts = [xts_all[ct] for ct in cts]
            hs = [
                [
                    hpool.tile([P, CT], MMD, name=f"h_{ct}_{ht}", tag="h")
                    for ht in range(KH)
                ]
                for ct in cts
            ]

            # ---- stage A: gate/up matmuls + silu*mul -> h ----
            for ht4 in range(NH4):
                gt, ut = gts[ht4], uts[ht4]
                for ci in range(len(cts)):
                    tmps = []
                    for sub in range(4):
                        ht = ht4 * 4 + sub
                        pg = pspool.tile([P, CT], F32, name=f"pg_{ht}_{ci}", tag="ps")
                        for kt in range(KD):
                            nc.tensor.matmul(
                                pg[:, :],
                                gt[:, kt, sub * P:(sub + 1) * P],
                                xts[ci][:, kt, :],
                                start=(kt == 0),
                                stop=(kt == KD - 1),
                            )
                        tmp = tpool.tile([P, CT], MMD, name=f"t_{ht}_{ci}", tag="t")
                        nc.scalar.activation(tmp[:, :], pg[:, :], AF.Silu)
                        tmps.append(tmp)
                    for sub in range(4):
                        ht = ht4 * 4 + sub
                        pu = pspool.tile([P, CT], F32, name=f"pu_{ht}_{ci}", tag="ps")
                        for kt in range(KD):
                            nc.tensor.matmul(
                                pu[:, :],
                                ut[:, kt, sub * P:(sub + 1) * P],
                                xts[ci][:, kt, :],
                                start=(kt == 0),
                                stop=(kt == KD - 1),
                            )
                        nc.vector.tensor_mul(
                            hs[ci][ht][:, :], tmps[sub][:, :], pu[:, :]
                        )

            # ---- stage B: down matmuls + prob scale -> y ----
            for dc in range(2):
                pos = {}
                for ci in range(len(cts)):
                    for m in range(CT // P):
                        pos[(ci, m)] = pspool.tile(
                            [P, 512], F32, name=f"po_{dc}_{ci}_{m}", tag="ps"
                        )
                ots = [
                    opool.tile([P, CT // P, 512], F32, name=f"o_{dc}_{ci}", tag="o")
                    for ci in range(len(cts))
                ]
                for hf in range(4):
                    dt_ = dts[(dc, hf)]
                    for kb in range(KH // 4):
                        kh = hf * (KH // 4) + kb
                        for ci in range(len(cts)):
                            for m in range(CT // P):
                                nc.tensor.matmul(
                                    pos[(ci, m)][:, :],
                                    hs[ci][kh][:, m * P:(m + 1) * P],
                                    dt_[:, kb, :],
                                    start=(kh == 0),
                                    stop=(kh == KH - 1),
                                )
                for ci, ct in enumerate(cts):
                    for m in range(CT // P):
                        j = ct * (CT // P) + m
                        nc.scalar.mul(
                            ots[ci][:, m, :], pos[(ci, m)][:, :], p_sb[:, j:j + 1]
                        )
                        q = nc.gpsimd if m % 2 == 0 else nc.scalar
                        q.dma_start(y[ct, dc][:, m, :], ots[ci][:, m, :])


def emit_expert_ffn_v4(tc, xt, gw, uw, dw, pv, y, MMD=None, warm=None, parts="full", store_q="split"):
    """v4: v2's just-in-time DMA schedule + v3's weight residency. Weights
    load once per rep at the same program positions as v2 (spread issue, no
    bandwidth burst) into static tiles; superblock 1 then runs with zero
    input DMAs. Tokens all hoisted on gpsimd; warmup matmuls bridge the
    head; y stores split across gpsimd/scalar queues."""
    MMD = MMD or MM_DTYPES[MM][0]
    nc = tc.nc
    sbs = [list(range(s, min(s + 2, NCT))) for s in range(0, NCT, 2)]

    with (
        tc.tile_pool(name="xpool", bufs=NCT) as xpool,
        tc.tile_pool(name="gpool", bufs=NH4) as gpool,
        tc.tile_pool(name="upool", bufs=NH4) as upool,
        tc.tile_pool(name="dwpool", bufs=8) as dwpool,
        tc.tile_pool(name="hpool", bufs=36) as hpool,
        tc.tile_pool(name="tpool", bufs=5) as tpool,
        tc.tile_pool(name="opool", bufs=2) as opool,
        tc.tile_pool(name="ppool", bufs=1) as ppool,
        tc.tile_pool(name="pspool", bufs=8, space="PSUM") as pspool,
    ):
        p_sb = ppool.tile([P, C // P], F32)
        nc.gpsimd.dma_start(p_sb[:, :], pv[:, :])

        # tokens up front on gpsimd (ct0 at quarter granularity)
        xts_all = []
        for ct in range(NCT):
            x_t = xpool.tile([P, KD, CT], MMD, name=f"xt_{ct}", tag="xt")
            if ct == 0:
                for q in range(4):
                    nc.gpsimd.dma_start(
                        x_t[:, 2 * q:2 * q + 2, :], xt[ct][:, 2 * q:2 * q + 2, :]
                    )
            else:
                nc.gpsimd.dma_start(x_t[:, 0:4, :], xt[ct][:, 0:4, :])
                nc.gpsimd.dma_start(x_t[:, 4:8, :], xt[ct][:, 4:8, :])
            xts_all.append(x_t)

        if warm is not None:
            wps = pspool.tile([P, 512], F32, name="warm_ps", tag="ps")
            for _ in range(12):
                nc.tensor.matmul(
                    wps[:, :], warm[:, 0:P], warm[:, :], start=True, stop=True
                )

        gts, uts, dts = [None] * NH4, [None] * NH4, {}

        for cts in sbs:
            first_sb = cts[0] == 0
            xts = [xts_all[ct] for ct in cts]
            hs = [
                [
                    hpool.tile([P, CT], MMD, name=f"h_{ct}_{ht}", tag="h")
                    for ht in range(KH)
                ]
                for ct in cts
            ]

            # ---- stage A ----
            for ht4 in range(NH4):
                if first_sb:
                    gt = gpool.tile([P, KD, 512], MMD, name=f"g_{ht4}", tag="gw")
                    if ht4 == 0:
                        for q in range(4):
                            nc.sync.dma_start(
                                gt[:, 2 * q:2 * q + 2, :],
                                gw[ht4][:, 2 * q:2 * q + 2, :],
                            )
                    else:
                        nc.sync.dma_start(gt[:, 0:4, :], gw[ht4][:, 0:4, :])
                        nc.sync.dma_start(gt[:, 4:8, :], gw[ht4][:, 4:8, :])
                    ut = upool.tile([P, KD, 512], MMD, name=f"u_{ht4}", tag="uw")
                    nc.scalar.dma_start(ut[:, 0:4, :], uw[ht4][:, 0:4, :])
                    nc.scalar.dma_start(ut[:, 4:8, :], uw[ht4][:, 4:8, :])
                    gts[ht4], uts[ht4] = gt, ut
                else:
                    gt, ut = gts[ht4], uts[ht4]
                for ci in range(len(cts)):
                    tmps = []
                    for sub in range(4):
                        ht = ht4 * 4 + sub
                        pg = pspool.tile([P, CT], F32, name=f"pg_{ht}_{ci}", tag="ps")
                        for kt in range(KD):
                            nc.tensor.matmul(
                                pg[:, :],
                                gt[:, kt, sub * P:(sub + 1) * P],
                                xts[ci][:, kt, :],
                                start=(kt == 0),
                                stop=(kt == KD - 1),
                            )
                        if parts != "a":
                            tmp = tpool.tile([P, CT], MMD, name=f"t_{ht}_{ci}", tag="t")
                            nc.scalar.activation(tmp[:, :], pg[:, :], AF.Silu)
                            tmps.append(tmp)
                    for sub in range(4):
                        ht = ht4 * 4 + sub
                        pu = pspool.tile([P, CT], F32, name=f"pu_{ht}_{ci}", tag="ps")
                        for kt in range(KD):
                            nc.tensor.matmul(
                                pu[:, :],
                                ut[:, kt, sub * P:(sub + 1) * P],
                                xts[ci][:, kt, :],
                                start=(kt == 0),
                                stop=(kt == KD - 1),
                            )
                        if parts != "a":
                            nc.vector.tensor_mul(
                                hs[ci][ht][:, :], tmps[sub][:, :], pu[:, :]
                            )

            # ---- stage B ----
            if parts in ("a", "b"):
                continue
            for dc in range(2):
                pos = {}
                for ci in range(len(cts)):
                    for m in range(CT // P):
                        pos[(ci, m)] = pspool.tile(
                            [P, 512], F32, name=f"po_{dc}_{ci}_{m}", tag="ps"
                        )
                ots = [
                    opool.tile([P, CT // P, 512], F32, name=f"o_{dc}_{ci}", tag="o")
                    for ci in range(len(cts))
                ]
                for hf in range(4):
                    if first_sb:
                        dt_ = dwpool.tile(
                            [P, KH // 4, 512], MMD, name=f"d_{dc}_{hf}", tag="dw"
                        )
                        nc.sync.dma_start(
                            dt_[:, :, :],
                            dw[dc, hf // 2][:, (hf % 2) * 4:(hf % 2) * 4 + 4, :],
                        )
                        dts[(dc, hf)] = dt_
                    else:
                        dt_ = dts[(dc, hf)]
                    for kb in range(KH // 4):
                        kh = hf * (KH // 4) + kb
                        for ci in range(len(cts)):
                            for m in range(CT // P):
                                nc.tensor.matmul(
                                    pos[(ci, m)][:, :],
                                    hs[ci][kh][:, m * P:(m + 1) * P],
                                    dt_[:, kb, :],
                                    start=(kh == 0),
                                    stop=(kh == KH - 1),
                                )
                for ci, ct in enumerate(cts):
                    for m in range(CT // P):
                        j = ct * (CT // P) + m
                        nc.scalar.mul(
                            ots[ci][:, m, :], pos[(ci, m)][:, :], p_sb[:, j:j + 1]
                        )
                        if store_q == "split":
                            q = nc.gpsimd if m % 2 == 0 else nc.scalar
                        else:
                            q = nc.gpsimd
                        q.dma_start(y[ct, dc][:, m, :], ots[ci][:, m, :])


def strip_pe_incs(nc):
    """Drop PE counting-sem increments that no wait ever targets (Tile's
    optimize_sems is disabled upstream; every matmul incs the PE sem, an
    EVT_SEM write costing ~26ns on the engine, 1536/rep). Keep exactly the
    increments whose cumulative count appears as some wait threshold (plus
    the final one) and renumber thresholds — each wait still becomes
    satisfied at the completion of the exact same matmul as before, so the
    transform is semantics-preserving (no rounding, no added delay, no new
    dependency cycles).
    """
    fn = nc.m.functions[0]
    insts = [inst for bb in fn.blocks for inst in bb.instructions]
    # Locate the PE counting sem: the one matmuls inc.
    pe_id = None
    for inst in insts:
        if isinstance(inst, mybir.InstMatmult) and inst.sync_info:
            for u in inst.sync_info.on_update:
                if u.sync_type == "semaphore" and u.update_mode == "sem-inc":
                    pe_id = u.id
                    break
        if pe_id is not None:
            break
    if pe_id is None:
        return
    # Enumerate inc events on that sem in program order.
    events = []  # instructions that sem-inc the PE sem, program order
    for inst in insts:
        si = inst.sync_info
        if not si:
            continue
        for u in si.on_update:
            if u.sync_type == "semaphore" and u.id == pe_id:
                if u.update_mode != "sem-inc" or u.update_value != 1:
                    continue  # e.g. the For_i reset's sem-set; leave as-is
                events.append(inst)
    if not events:
        return
    # Collect every wait threshold on the sem.
    targets = set()
    pe_waits = []
    for inst in insts:
        si = inst.sync_info
        if not si:
            continue
        for w in si.on_wait:
            if (
                w.sync_type == "semaphore"
                and w.id == pe_id
                and w.wait_mode == "sem-ge-imm"
                and w.wait_value > 0  # >=0 waits (skip path) are no-ops
            ):
                assert w.wait_value <= len(events)
                targets.add(w.wait_value)
                pe_waits.append(w)
    kept = sorted(targets | {len(events)})
    rank = {old: i + 1 for i, old in enumerate(kept)}
    for w in pe_waits:
        w.wait_value = rank[w.wait_value]
    # The For_i reset adjusts the sem by the old per-iteration total
    # (sem-add-imm/sem-sub-imm 1536) — rescale to the kept count.
    for inst in insts:
        si = inst.sync_info
        if not si:
            continue
        for u in si.on_update:
            if (
                u.sync_type == "semaphore"
                and u.id == pe_id
                and u.update_mode in ("sem-add-imm", "sem-sub-imm")
            ):
                assert u.update_value == len(events), (
                    f"unexpected bulk sem adjust {u.update_mode} "
                    f"{u.update_value} != {len(events)}"
                )
                u.update_value = len(kept)
    keep_set = set(kept)
    for i, inst in enumerate(events, 1):
        if i in keep_set:
            continue
        si = inst.sync_info
        new_upd = [
            u
            for u in si.on_update
            if not (u.sync_type == "semaphore" and u.id == pe_id)
        ]
        inst.sync_info = mybir.SyncInfo(
            on_wait=list(si.on_wait), on_update=new_upd
        )


def build_nc(reps_loop=False, max_reps=512, mm=None, strip=True, ver=4, parts="full", store_q="split"):
    mmd = MM_DTYPES[mm or MM][0]
    """Build the per-core Bass program. With reps_loop, the whole body runs
    inside a For_i whose trip count is read from an int32 input "reps"."""
    nc = bacc.Bacc(None, target_bir_lowering=False)
    with tile.TileContext(nc) as tc:
        xt = nc.dram_tensor("xt", [NCT, P, KD, CT], mmd, kind="ExternalInput")
        gw = nc.dram_tensor("gw", [NH4, P, KD, 512], mmd, kind="ExternalInput")
        uw = nc.dram_tensor("uw", [NH4, P, KD, 512], mmd, kind="ExternalInput")
        dw = nc.dram_tensor("dw", [2, 2, P, KH // 2, 512], mmd, kind="ExternalInput")
        pv = nc.dram_tensor("pv", [P, C // P], F32, kind="ExternalInput")
        y = nc.dram_tensor("y", [NCT, 2, P, CT // P, 512], F32, kind="ExternalOutput")
        if ver == 4:
            def emit(warm):
                emit_expert_ffn_v4(tc, xt, gw, uw, dw, pv, y, MMD=mmd, warm=warm, parts=parts, store_q=store_q)
        elif ver == 3:
            def emit(warm):
                emit_expert_ffn_v3(tc, xt, gw, uw, dw, pv, y, MMD=mmd, warm=warm)
        elif ver == 2:
            def emit(warm):
                emit_expert_ffn_v2(tc, xt, gw, uw, dw, pv, y, MMD=mmd, warm=warm)
        else:
            def emit(warm):
                emit_expert_ffn(tc, xt, gw, uw, dw, pv, y, MMD=mmd)
        if reps_loop:
            reps = nc.dram_tensor("reps", [1, 1], mybir.dt.int32, kind="ExternalInput")
            with tc.tile_pool(name="rpool", bufs=1) as rpool, \
                 tc.tile_pool(name="spool", bufs=1) as spool:
                r_sb = rpool.tile([1, 1], mybir.dt.int32)
                nc.sync.dma_start(r_sb[:, :], reps[:, :])
                rv = nc.values_load(
                    r_sb[0:1, 0:1],
                    min_val=0,
                    max_val=max_reps,
                    skip_runtime_bounds_check=True,
                )
                warm = None
                if ver >= 2:
                    warm = spool.tile([P, 512], mmd, name="warm")
                    nc.vector.memset(warm[:, :], 0.0)
                with tc.For_i(0, rv, 1):
                    emit(warm)
        else:
            with tc.tile_pool(name="spool", bufs=1) as spool:
                warm = None
                if ver >= 2:
                    warm = spool.tile([P, 512], mmd, name="warm")
                    nc.vector.memset(warm[:, :], 0.0)
                emit(warm)
    nc.compile()
    # strip AFTER compile: the compile passes (move_matmul_waits_to_ldweights,
    # generate_event_semaphores, loop lowering) re-derive sem totals, so
    # rewriting before them leaves stale counts behind
    if strip:
        strip_pe_incs(nc)
    return nc


def pack_inputs(x_pad, gate_w_e, up_w_e, down_w_e, p_pad, mm=None):
    npdt = MM_DTYPES[mm or MM][1]
    """Pack one expert's inputs into the SBUF-tile-order DRAM layouts.
    Matmul operands are cast to bf16 (cast first: halves the transpose
    bytes)."""
    # xt [NCT, 128, KD, 512]: [ct, p, kt, tok] = x_pad[ct*512+tok, kt*128+p]
    xt = np.ascontiguousarray(
        x_pad.astype(npdt).reshape(NCT, CT, KD, P).transpose(0, 3, 2, 1)
    )
    # gw/uw [NH4, 128, KD, 512]: [b, p, kt, h] = w[b*512+h, kt*128+p]
    gw = np.ascontiguousarray(
        gate_w_e.astype(npdt).reshape(NH4, 512, KD, P).transpose(0, 3, 2, 1)
    )
    uw = np.ascontiguousarray(
        up_w_e.astype(npdt).reshape(NH4, 512, KD, P).transpose(0, 3, 2, 1)
    )
    # dw [2, 2, 128, KH//2, 512]: [dc, hf, p, kb, d] = down[dc*512+d, hf*1024+kb*128+p]
    dw = np.ascontiguousarray(
        down_w_e.astype(npdt).reshape(2, 512, 2, KH // 2, P).transpose(0, 2, 4, 3, 1)
    )
    pv = np.ascontiguousarray(p_pad.reshape(C // P, P).T)
    return {"xt": xt, "gw": gw, "uw": uw, "dw": dw, "pv": pv}


def unpack_y(y_pack):
    """y_pack [NCT, 2, 128, 4, 512] -> y [C, D]."""
    return np.ascontiguousarray(
        y_pack.transpose(0, 3, 2, 1, 4).reshape(C, D)
    )


def route_and_dispatch(x, router_w):
    """Host router + top-2 dispatch (matches softmax/top_k/renorm of the
    reference exactly)."""
    logits = x @ router_w.T                      # [T, E]
    t_ar = np.arange(T)
    i1 = np.argmax(logits, axis=1)
    l1 = logits[t_ar, i1]
    lm = logits.copy()
    lm[t_ar, i1] = -np.inf
    i2 = np.argmax(lm, axis=1)
    l2 = lm[t_ar, i2]
    e2 = np.exp(l2 - l1)
    p1 = 1.0 / (1.0 + e2)
    p2 = e2 / (1.0 + e2)

    ee = np.concatenate([i1, i2])                # [2T] expert of each pair
    tt = np.concatenate([t_ar, t_ar])            # [2T] token of each pair
    pp = np.concatenate([p1, p2]).astype(np.float32)
    counts = np.bincount(ee, minlength=E)
    starts = np.zeros(E, np.int64)
    starts[1:] = np.cumsum(counts)[:-1]
    order = np.argsort(ee, kind="stable")
    pos = np.empty(2 * T, np.int64)
    pos[order] = np.arange(2 * T) - starts[ee[order]]
    return ee, tt, pp, pos, counts, starts, order


def kernel(**inputs):
    x = np.ascontiguousarray(
        np.asarray(inputs["hidden_states"], np.float32).reshape(T, D)
    )
    router_w = np.asarray(inputs["router_w"], np.float32)
    gate_w = np.asarray(inputs["gate_w"], np.float32)
    up_w = np.asarray(inputs["up_w"], np.float32)
    down_w = np.asarray(inputs["down_w"], np.float32)

    ee, tt, pp, pos, counts, starts, order = route_and_dispatch(x, router_w)

    in_maps = []
    for e in range(E):
        n_e = min(int(counts[e]), C)
        sel = order[starts[e]:starts[e] + n_e]   # pairs dispatched to core e
        xp = np.zeros((C, D), np.float32)
        xp[:n_e] = x[tt[sel]]
        pvec = np.zeros(C, np.float32)
        pvec[:n_e] = pp[sel]
        in_maps.append(pack_inputs(xp, gate_w[e], up_w[e], down_w[e], pvec))

    nc = build_nc()
    res = run_bass_kernel_spmd(nc, in_maps, core_ids=list(range(E)))
    ys = np.stack(
        [unpack_y(res.results[e]["y"]) for e in range(E)]
    ).reshape(E * C, D)

    ok = pos < C
    contrib = np.zeros((2 * T, D), np.float32)
    g = ee * C + pos
    contrib[ok] = ys[g[ok]]
    # capacity-overflow fallback: exact fp32 host compute for the few pairs
    # beyond capacity (~0.8% of pairs for the seed-0 routing), batched per
    # expert
    if not ok.all():
        bad = np.nonzero(~ok)[0]
        for e in np.unique(ee[bad]):
            js = bad[ee[bad] == e]
            xb = x[tt[js]]
            gb = xb @ gate_w[e].T
            ub = xb @ up_w[e].T
            hb = (gb / (1.0 + np.exp(-gb))) * ub
            contrib[js] = (hb @ down_w[e].T) * pp[js, None]
    out = contrib[:T] + contrib[T:]
    return out.reshape(B, S, D).astype(np.float32)



# revision 27
# speedup vs baseline: 1.1697x; 1.1697x over previous
"""MoE layer (top-2 of 8 experts) for 8 Trainium2 NeuronCores.

Strategy: expert-parallel. Host computes the (tiny) router + top-2 dispatch in
numpy; core e runs expert e's FFN over its dispatched tokens (padded to a fixed
capacity C) with bf16 matmuls (fp32 PSUM accumulation); host combines the two
expert outputs per token.

All device matmuls are [K=128]x[M=128]x[N=512] bf16 (1 cycle/row):
  gate^T/up^T [H, Ct] = gwT/uwT.T @ xt   (contraction over D, 8 k-tiles)
  h = silu(gate) * up                    (SBUF-resident [128, 512] tiles)
  y [Ct, D] = (h.T @ dwT) * p            (contraction over H, 16 k-tiles,
                                          combine-prob scale fused in eviction)

The host pre-packs weights and tokens into SBUF-tile order so every load is a
large DMA with contiguous per-partition lines. v4 schedule: weights are
SBUF-resident (loaded once per invocation at just-in-time program positions),
tokens all hoisted to the start, PE warmup matmuls on a zeroed scratch tile
bridge the initial DMA wait, and redundant per-matmul PE semaphore increments
are stripped post-compile (strip_pe_incs).
"""

import ml_dtypes
import numpy as np

import concourse.bass as bass
import concourse.mybir as mybir
import concourse.tile as tile
from concourse import bacc
from concourse.bass_utils import run_bass_kernel_spmd

E = 8
TOP_K = 2
B, S, D, H = 4, 2048, 1024, 2048
T = B * S
C = 2048          # per-expert token capacity; overflow pairs (seed-0: ~137
                  # of 16384, counts max 2175) fall back to exact host compute
CT = 512          # token tile
P = 128
NCT = C // CT     # 4
KD = D // P       # 8  k-tiles for gate/up
KH = H // P       # 16 k-tiles for down
NH4 = H // 512    # 4  groups of 4 h-blocks
F32 = mybir.dt.float32
F32R = mybir.dt.float32r
# matmul operand dtype: bf16 streams at the same 1 cycle/row as fp32r, but
# enables FWL + LDWEIGHTS pull-ahead (fp32 weight loads serialize inside the
# self-loading matmul) and halves DMA traffic. PSUM accumulation stays fp32.
BF16 = mybir.dt.bfloat16
NP_BF16 = ml_dtypes.bfloat16
MM_DTYPES = {"bf16": (BF16, NP_BF16), "f32r": (F32R, np.float32)}
MM = "bf16"   # default matmul-operand dtype
AF = mybir.ActivationFunctionType


def emit_expert_ffn(tc, xt, gw, uw, dw, pv, y, MMD=None):
    MMD = MMD or MM_DTYPES[MM][0]
    """Emit one expert's FFN.

    DRAM tensors (all pre-packed on host):
      xt [NCT, 128, KD, 512] f32r - tokens, transposed per ct tile
      gw/uw [NH4, 128, KD, 512] f32r - gate/up weights per 4-h-block group
      dw [2, 2, 128, KH//2, 512] f32r - down weights per (dc, kh-half)
      pv [128, C//128] f32 - combine probs (token-partition layout)
      y  [NCT, 2, 128, 4, 512] f32 out - [ct, dc, p, m, 512]
    """
    nc = tc.nc
    # superblocks of up to 2 token tiles sharing one weight pass
    sbs = [list(range(s, min(s + 2, NCT))) for s in range(0, NCT, 2)]

    with (
        tc.tile_pool(name="xpool", bufs=2) as xpool,
        tc.tile_pool(name="wpool", bufs=3) as wpool,
        tc.tile_pool(name="hpool", bufs=36) as hpool,
        tc.tile_pool(name="dpool", bufs=3) as dpool,
        tc.tile_pool(name="tpool", bufs=5) as tpool,
        tc.tile_pool(name="opool", bufs=2) as opool,
        tc.tile_pool(name="ppool", bufs=1) as ppool,
        tc.tile_pool(name="pspool", bufs=8, space="PSUM") as pspool,
    ):
        p_sb = ppool.tile([P, C // P], F32)
        nc.gpsimd.dma_start(p_sb[:, :], pv[:, :])

        for cts in sbs:
            # ---- token tiles: one 2MB DMA per ct ----
            xts = []
            for ct in cts:
                x_t = xpool.tile([P, KD, CT], MMD, name=f"xt_{ct}", tag="xt")
                nc.gpsimd.dma_start(x_t[:, 0:4, :], xt[ct][:, 0:4, :])
                nc.gpsimd.dma_start(x_t[:, 4:8, :], xt[ct][:, 4:8, :])
                xts.append(x_t)
            hs = [
                [
                    hpool.tile([P, CT], MMD, name=f"h_{ct}_{ht}", tag="h")
                    for ht in range(KH)
                ]
                for ct in cts
            ]

            # ---- stage A: gate/up matmuls + silu*mul -> h ----
            first_sb = cts[0] == 0
            for ht4 in range(NH4):
                gt = wpool.tile([P, KD, 512], MMD, name=f"g_{ht4}", tag="w")
                if ht4 == 0 and first_sb:
                    # quarter-granularity on the very first load so the first
                    # matmuls start ~2us earlier out of the cold start
                    for q in range(4):
                        nc.sync.dma_start(
                            gt[:, 2 * q:2 * q + 2, :], gw[ht4][:, 2 * q:2 * q + 2, :]
                        )
                else:
                    nc.sync.dma_start(gt[:, 0:4, :], gw[ht4][:, 0:4, :])
                    nc.sync.dma_start(gt[:, 4:8, :], gw[ht4][:, 4:8, :])
                ut = wpool.tile([P, KD, 512], MMD, name=f"u_{ht4}", tag="w")
                nc.scalar.dma_start(ut[:, 0:4, :], uw[ht4][:, 0:4, :])
                nc.scalar.dma_start(ut[:, 4:8, :], uw[ht4][:, 4:8, :])
                # ct-major, all-gate-then-all-up: gt's last use lands at ~75%
                # of the group so the next group's weight DMA overlaps compute
                for ci in range(len(cts)):
                    tmps = []
                    for sub in range(4):
                        ht = ht4 * 4 + sub
                        pg = pspool.tile([P, CT], F32, name=f"pg_{ht}_{ci}", tag="ps")
                        for kt in range(KD):
                            nc.tensor.matmul(
                                pg[:, :],
                                gt[:, kt, sub * P:(sub + 1) * P],
                                xts[ci][:, kt, :],
                                start=(kt == 0),
                                stop=(kt == KD - 1),
                            )
                        tmp = tpool.tile([P, CT], MMD, name=f"t_{ht}_{ci}", tag="t")
                        nc.scalar.activation(tmp[:, :], pg[:, :], AF.Silu)
                        tmps.append(tmp)
                    for sub in range(4):
                        ht = ht4 * 4 + sub
                        pu = pspool.tile([P, CT], F32, name=f"pu_{ht}_{ci}", tag="ps")
                        for kt in range(KD):
                            nc.tensor.matmul(
                                pu[:, :],
                                ut[:, kt, sub * P:(sub + 1) * P],
                                xts[ci][:, kt, :],
                                start=(kt == 0),
                                stop=(kt == KD - 1),
                            )
                        nc.vector.tensor_mul(
                            hs[ci][ht][:, :], tmps[sub][:, :], pu[:, :]
                        )

            # ---- stage B: down matmuls + prob scale -> y ----
            for dc in range(2):
                pos = {}
                for ci in range(len(cts)):
                    for m in range(CT // P):
                        pos[(ci, m)] = pspool.tile(
                            [P, 512], F32, name=f"po_{dc}_{ci}_{m}", tag="ps"
                        )
                ots = [
                    opool.tile([P, CT // P, 512], F32, name=f"o_{dc}_{ci}", tag="o")
                    for ci in range(len(cts))
                ]
                for hf in range(4):
                    dt_ = dpool.tile([P, KH // 4, 512], MMD, name=f"d_{dc}_{hf}", tag="dw")
                    nc.sync.dma_start(
                        dt_[:, :, :], dw[dc, hf // 2][:, (hf % 2) * 4:(hf % 2) * 4 + 4, :]
                    )
                    for kb in range(KH // 4):
                        kh = hf * (KH // 4) + kb
                        for ci in range(len(cts)):
                            for m in range(CT // P):
                                nc.tensor.matmul(
                                    pos[(ci, m)][:, :],
                                    hs[ci][kh][:, m * P:(m + 1) * P],
                                    dt_[:, kb, :],
                                    start=(kh == 0),
                                    stop=(kh == KH - 1),
                                )
                for ci, ct in enumerate(cts):
                    for m in range(CT // P):
                        j = ct * (CT // P) + m
                        nc.scalar.mul(
                            ots[ci][:, m, :], pos[(ci, m)][:, :], p_sb[:, j:j + 1]
                        )
                        # per-m stores start as soon as each eviction lands,
                        # shortening the kernel-tail drain
                        nc.gpsimd.dma_start(y[ct, dc][:, m, :], ots[ci][:, m, :])


def emit_expert_ffn_v2(tc, xt, gw, uw, dw, pv, y, MMD=None, warm=None):
    """v2: all xt loads hoisted to rep start on the vector DMA queue (first
    ct at quarter granularity for a fast cold start), PE warmup matmuls on a
    zero scratch tile to bridge the head DMA wait (keeps HAM at K=8/8 across
    For_i reps), y stores split across the gpsimd and vector queues."""
    MMD = MMD or MM_DTYPES[MM][0]
    nc = tc.nc
    sbs = [list(range(s, min(s + 2, NCT))) for s in range(0, NCT, 2)]

    with (
        tc.tile_pool(name="xpool", bufs=4) as xpool,
        tc.tile_pool(name="wpool", bufs=3) as wpool,
        tc.tile_pool(name="hpool", bufs=36) as hpool,
        tc.tile_pool(name="dpool", bufs=3) as dpool,
        tc.tile_pool(name="tpool", bufs=5) as tpool,
        tc.tile_pool(name="opool", bufs=2) as opool,
        tc.tile_pool(name="ppool", bufs=1) as ppool,
        tc.tile_pool(name="pspool", bufs=8, space="PSUM") as pspool,
    ):
        p_sb = ppool.tile([P, C // P], F32)
        nc.gpsimd.dma_start(p_sb[:, :], pv[:, :])

        # all token tiles up front on the vector queue
        xts_all = []
        for ct in range(NCT):
            x_t = xpool.tile([P, KD, CT], MMD, name=f"xt_{ct}", tag="xt")
            if ct == 0:
                for q in range(4):
                    nc.gpsimd.dma_start(
                        x_t[:, 2 * q:2 * q + 2, :], xt[ct][:, 2 * q:2 * q + 2, :]
                    )
            else:
                nc.gpsimd.dma_start(x_t[:, 0:4, :], xt[ct][:, 0:4, :])
                nc.gpsimd.dma_start(x_t[:, 4:8, :], xt[ct][:, 4:8, :])
            xts_all.append(x_t)

        # PE warmup: dummy matmuls on the preamble-zeroed scratch tile; no
        # DMA dependencies, so the PE chews these while the first loads land
        if warm is not None:
            wps = pspool.tile([P, 512], F32, name="warm_ps", tag="ps")
            for _ in range(12):
                nc.tensor.matmul(
                    wps[:, :], warm[:, 0:P], warm[:, :], start=True, stop=True
                )

        for cts in sbs:
            xts = [xts_all[ct] for ct in cts]
            hs = [
                [
                    hpool.tile([P, CT], MMD, name=f"h_{ct}_{ht}", tag="h")
                    for ht in range(KH)
                ]
                for ct in cts
            ]

            # ---- stage A: gate/up matmuls + silu*mul -> h ----
            first_sb = cts[0] == 0
            for ht4 in range(NH4):
                gt = wpool.tile([P, KD, 512], MMD, name=f"g_{ht4}", tag="w")
                if ht4 == 0 and first_sb:
                    for q in range(4):
                        nc.sync.dma_start(
                            gt[:, 2 * q:2 * q + 2, :], gw[ht4][:, 2 * q:2 * q + 2, :]
                        )
                else:
                    nc.sync.dma_start(gt[:, 0:4, :], gw[ht4][:, 0:4, :])
                    nc.sync.dma_start(gt[:, 4:8, :], gw[ht4][:, 4:8, :])
                ut = wpool.tile([P, KD, 512], MMD, name=f"u_{ht4}", tag="w")
                nc.scalar.dma_start(ut[:, 0:4, :], uw[ht4][:, 0:4, :])
                nc.scalar.dma_start(ut[:, 4:8, :], uw[ht4][:, 4:8, :])
                for ci in range(len(cts)):
                    tmps = []
                    for sub in range(4):
                        ht = ht4 * 4 + sub
                        pg = pspool.tile([P, CT], F32, name=f"pg_{ht}_{ci}", tag="ps")
                        for kt in range(KD):
                            nc.tensor.matmul(
                                pg[:, :],
                                gt[:, kt, sub * P:(sub + 1) * P],
                                xts[ci][:, kt, :],
                                start=(kt == 0),
                                stop=(kt == KD - 1),
                            )
                        tmp = tpool.tile([P, CT], MMD, name=f"t_{ht}_{ci}", tag="t")
                        nc.scalar.activation(tmp[:, :], pg[:, :], AF.Silu)
                        tmps.append(tmp)
                    for sub in range(4):
                        ht = ht4 * 4 + sub
                        pu = pspool.tile([P, CT], F32, name=f"pu_{ht}_{ci}", tag="ps")
                        for kt in range(KD):
                            nc.tensor.matmul(
                                pu[:, :],
                                ut[:, kt, sub * P:(sub + 1) * P],
                                xts[ci][:, kt, :],
                                start=(kt == 0),
                                stop=(kt == KD - 1),
                            )
                        nc.vector.tensor_mul(
                            hs[ci][ht][:, :], tmps[sub][:, :], pu[:, :]
                        )

            # ---- stage B: down matmuls + prob scale -> y ----
            for dc in range(2):
                pos = {}
                for ci in range(len(cts)):
                    for m in range(CT // P):
                        pos[(ci, m)] = pspool.tile(
                            [P, 512], F32, name=f"po_{dc}_{ci}_{m}", tag="ps"
                        )
                ots = [
                    opool.tile([P, CT // P, 512], F32, name=f"o_{dc}_{ci}", tag="o")
                    for ci in range(len(cts))
                ]
                for hf in range(4):
                    dt_ = dpool.tile([P, KH // 4, 512], MMD, name=f"d_{dc}_{hf}", tag="dw")
                    nc.sync.dma_start(
                        dt_[:, :, :], dw[dc, hf // 2][:, (hf % 2) * 4:(hf % 2) * 4 + 4, :]
                    )
                    for kb in range(KH // 4):
                        kh = hf * (KH // 4) + kb
                        for ci in range(len(cts)):
                            for m in range(CT // P):
                                nc.tensor.matmul(
                                    pos[(ci, m)][:, :],
                                    hs[ci][kh][:, m * P:(m + 1) * P],
                                    dt_[:, kb, :],
                                    start=(kh == 0),
                                    stop=(kh == KH - 1),
                                )
                for ci, ct in enumerate(cts):
                    for m in range(CT // P):
                        j = ct * (CT // P) + m
                        nc.scalar.mul(
                            ots[ci][:, m, :], pos[(ci, m)][:, :], p_sb[:, j:j + 1]
                        )
                        q = nc.gpsimd if m % 2 == 0 else nc.scalar
                        q.dma_start(y[ct, dc][:, m, :], ots[ci][:, m, :])


def emit_expert_ffn_v3(tc, xt, gw, uw, dw, pv, y, MMD=None, warm=None):
    """v3: fully SBUF-resident weights/tokens (bf16 fits: ~186KB/partition).
    All input DMAs issue up front, ordered by first-use time across the three
    queues, so no matmul ever waits on a mid-rep load; the second superblock
    runs with zero input DMAs. Warmup matmuls bridge the initial load."""
    MMD = MMD or MM_DTYPES[MM][0]
    nc = tc.nc
    sbs = [list(range(s, min(s + 2, NCT))) for s in range(0, NCT, 2)]

    with (
        tc.tile_pool(name="xpool", bufs=NCT) as xpool,
        tc.tile_pool(name="gpool", bufs=NH4) as gpool,
        tc.tile_pool(name="upool", bufs=NH4) as upool,
        tc.tile_pool(name="dwpool", bufs=8) as dwpool,
        tc.tile_pool(name="hpool", bufs=36) as hpool,
        tc.tile_pool(name="tpool", bufs=5) as tpool,
        tc.tile_pool(name="opool", bufs=2) as opool,
        tc.tile_pool(name="ppool", bufs=1) as ppool,
        tc.tile_pool(name="pspool", bufs=8, space="PSUM") as pspool,
    ):
        p_sb = ppool.tile([P, C // P], F32)
        nc.gpsimd.dma_start(p_sb[:, :], pv[:, :])

        # ---- all input loads up front, ordered by first use ----
        # gpsimd queue: tokens (ct0 at quarter granularity)
        xts_all = []
        for ct in range(NCT):
            x_t = xpool.tile([P, KD, CT], MMD, name=f"xt_{ct}", tag="xt")
            if ct == 0:
                for q in range(4):
                    nc.gpsimd.dma_start(
                        x_t[:, 2 * q:2 * q + 2, :], xt[ct][:, 2 * q:2 * q + 2, :]
                    )
            else:
                nc.gpsimd.dma_start(x_t[:, 0:4, :], xt[ct][:, 0:4, :])
                nc.gpsimd.dma_start(x_t[:, 4:8, :], xt[ct][:, 4:8, :])
            xts_all.append(x_t)
        # sync queue: gate weights then down dc1; scalar: up weights then dc0
        gts, uts = [], []
        for g in range(NH4):
            gt = gpool.tile([P, KD, 512], MMD, name=f"g_{g}", tag="gw")
            if g == 0:
                for q in range(4):
                    nc.sync.dma_start(
                        gt[:, 2 * q:2 * q + 2, :], gw[g][:, 2 * q:2 * q + 2, :]
                    )
            else:
                nc.sync.dma_start(gt[:, 0:4, :], gw[g][:, 0:4, :])
                nc.sync.dma_start(gt[:, 4:8, :], gw[g][:, 4:8, :])
            gts.append(gt)
        for g in range(NH4):
            ut = upool.tile([P, KD, 512], MMD, name=f"u_{g}", tag="uw")
            nc.scalar.dma_start(ut[:, 0:4, :], uw[g][:, 0:4, :])
            nc.scalar.dma_start(ut[:, 4:8, :], uw[g][:, 4:8, :])
            uts.append(ut)
        dts = {}
        for dc in range(2):
            for hf in range(4):
                dt_ = dwpool.tile(
                    [P, KH // 4, 512], MMD, name=f"d_{dc}_{hf}", tag="dw"
                )
                q = nc.scalar if dc == 0 else nc.sync
                q.dma_start(
                    dt_[:, :, :],
                    dw[dc, hf // 2][:, (hf % 2) * 4:(hf % 2) * 4 + 4, :],
                )
                dts[(dc, hf)] = dt_

        # PE warmup on the preamble-zeroed scratch: no DMA deps
        if warm is not None:
            wps = pspool.tile([P, 512], F32, name="warm_ps", tag="ps")
            for _ in range(12):
                nc.tensor.matmul(
                    wps[:, :], warm[:, 0:P], warm[:, :], start=True, stop=True
                )

        for cts in sbs:
            xts = [xts_all[ct] for ct in cts]
            hs = [
                [
                    hpool.tile([P, CT], MMD, name=f"h_{ct}_{ht}", tag="h")
                    for ht in range(KH)
                ]
                for ct in cts
            ]

            # ---- stage A: gate/up matmuls + silu*mul -> h ----
            for ht4 in range(NH4):
                gt, ut = gts[ht4], uts[ht4]
                for ci in range(len(cts)):
                    tmps = []
                    for sub in range(4):
                        ht = ht4 * 4 + sub
                        pg = pspool.tile([P, CT], F32, name=f"pg_{ht}_{ci}", tag="ps")
                        for kt in range(KD):
                            nc.tensor.matmul(
                                pg[:, :],
                                gt[:, kt, sub * P:(sub + 1) * P],
                                xts[ci][:, kt, :],
                                start=(kt == 0),
                                stop=(kt == KD - 1),
                            )
                        tmp = tpool.tile([P, CT], MMD, name=f"t_{ht}_{ci}", tag="t")
                        nc.scalar.activation(tmp[:, :], pg[:, :], AF.Silu)
                        tmps.append(tmp)
                    for sub in range(4):
                        ht = ht4 * 4 + sub
                        pu = pspool.tile([P, CT], F32, name=f"pu_{ht}_{ci}", tag="ps")
                        for kt in range(KD):
                            nc.tensor.matmul(
                                pu[:, :],
                                ut[:, kt, sub * P:(sub + 1) * P],
                                xts[ci][:, kt, :],
                                start=(kt == 0),
                                stop=(kt == KD - 1),
                            )
                        nc.vector.tensor_mul(
                            hs[ci][ht][:, :], tmps[sub][:, :], pu[:, :]
                        )

            # ---- stage B: down matmuls + prob scale -> y ----
            for dc in range(2):
                pos = {}
                for ci in range(len(cts)):
                    for m in range(CT // P):
                        pos[(ci, m)] = pspool.tile(
                            [P, 512], F32, name=f"po_{dc}_{ci}_{m}", tag="ps"
                        )
                ots = [
                    opool.tile([P, CT // P, 512], F32, name=f"o_{dc}_{ci}", tag="o")
                    for ci in range(len(cts))
                ]
                for hf in range(4):
                    dt_ = dts[(dc, hf)]
                    for kb in range(KH // 4):
                        kh = hf * (KH // 4) + kb
                        for ci in range(len(cts)):
                            for m in range(CT // P):
                                nc.tensor.matmul(
                                    pos[(ci, m)][:, :],
                                    hs[ci][kh][:, m * P:(m + 1) * P],
                                    dt_[:, kb, :],
                                    start=(kh == 0),
                                    stop=(kh == KH - 1),
                                )
                for ci, ct in enumerate(cts):
                    for m in range(CT // P):
                        j = ct * (CT // P) + m
                        nc.scalar.mul(
                            ots[ci][:, m, :], pos[(ci, m)][:, :], p_sb[:, j:j + 1]
                        )
                        q = nc.gpsimd if m % 2 == 0 else nc.scalar
                        q.dma_start(y[ct, dc][:, m, :], ots[ci][:, m, :])


def emit_expert_ffn_v4(tc, xt, gw, uw, dw, pv, y, MMD=None, warm=None, parts="full", store_q="split"):
    """v4: v2's just-in-time DMA schedule + v3's weight residency. Weights
    load once per rep at the same program positions as v2 (spread issue, no
    bandwidth burst) into static tiles; superblock 1 then runs with zero
    input DMAs. Tokens all hoisted on gpsimd; warmup matmuls bridge the
    head; y stores split across gpsimd/scalar queues."""
    MMD = MMD or MM_DTYPES[MM][0]
    nc = tc.nc
    sbs = [list(range(s, min(s + 2, NCT))) for s in range(0, NCT, 2)]

    with (
        tc.tile_pool(name="xpool", bufs=NCT) as xpool,
        tc.tile_pool(name="gpool", bufs=NH4) as gpool,
        tc.tile_pool(name="upool", bufs=NH4) as upool,
        tc.tile_pool(name="dwpool", bufs=8) as dwpool,
        tc.tile_pool(name="hpool", bufs=36) as hpool,
        tc.tile_pool(name="tpool", bufs=5) as tpool,
        tc.tile_pool(name="opool", bufs=2) as opool,
        tc.tile_pool(name="ppool", bufs=1) as ppool,
        tc.tile_pool(name="pspool", bufs=8, space="PSUM") as pspool,
    ):
        p_sb = ppool.tile([P, C // P], F32)
        nc.gpsimd.dma_start(p_sb[:, :], pv[:, :])

        # tokens up front on gpsimd (ct0 at quarter granularity)
        xts_all = []
        for ct in range(NCT):
            x_t = xpool.tile([P, KD, CT], MMD, name=f"xt_{ct}", tag="xt")
            if ct == 0:
                for q in range(4):
                    nc.gpsimd.dma_start(
                        x_t[:, 2 * q:2 * q + 2, :], xt[ct][:, 2 * q:2 * q + 2, :]
                    )
            else:
                nc.gpsimd.dma_start(x_t[:, 0:4, :], xt[ct][:, 0:4, :])
                nc.gpsimd.dma_start(x_t[:, 4:8, :], xt[ct][:, 4:8, :])
            xts_all.append(x_t)

        if warm is not None:
            wps = pspool.tile([P, 512], F32, name="warm_ps", tag="ps")
            for _ in range(12):
                nc.tensor.matmul(
                    wps[:, :], warm[:, 0:P], warm[:, :], start=True, stop=True
                )

        gts, uts, dts = [None] * NH4, [None] * NH4, {}

        for cts in sbs:
            first_sb = cts[0] == 0
            xts = [xts_all[ct] for ct in cts]
            hs = [
                [
                    hpool.tile([P, CT], MMD, name=f"h_{ct}_{ht}", tag="h")
                    for ht in range(KH)
                ]
                for ct in cts
            ]

            # ---- stage A ----
            for ht4 in range(NH4):
                if first_sb:
                    gt = gpool.tile([P, KD, 512], MMD, name=f"g_{ht4}", tag="gw")
                    if ht4 == 0:
                        for q in range(4):
                            nc.sync.dma_start(
                                gt[:, 2 * q:2 * q + 2, :],
                                gw[ht4][:, 2 * q:2 * q + 2, :],
                            )
                    else:
                        nc.sync.dma_start(gt[:, 0:4, :], gw[ht4][:, 0:4, :])
                        nc.sync.dma_start(gt[:, 4:8, :], gw[ht4][:, 4:8, :])
                    ut = upool.tile([P, KD, 512], MMD, name=f"u_{ht4}", tag="uw")
                    nc.scalar.dma_start(ut[:, 0:4, :], uw[ht4][:, 0:4, :])
                    nc.scalar.dma_start(ut[:, 4:8, :], uw[ht4][:, 4:8, :])
                    gts[ht4], uts[ht4] = gt, ut
                else:
                    gt, ut = gts[ht4], uts[ht4]
                for ci in range(len(cts)):
                    tmps = []
                    for sub in range(4):
                        ht = ht4 * 4 + sub
                        pg = pspool.tile([P, CT], F32, name=f"pg_{ht}_{ci}", tag="ps")
                        for kt in range(KD):
                            nc.tensor.matmul(
                                pg[:, :],
                                gt[:, kt, sub * P:(sub + 1) * P],
                                xts[ci][:, kt, :],
                                start=(kt == 0),
                                stop=(kt == KD - 1),
                            )
                        if parts != "a":
                            tmp = tpool.tile([P, CT], MMD, name=f"t_{ht}_{ci}", tag="t")
                            nc.scalar.activation(tmp[:, :], pg[:, :], AF.Silu)
                            tmps.append(tmp)
                    for sub in range(4):
                        ht = ht4 * 4 + sub
                        pu = pspool.tile([P, CT], F32, name=f"pu_{ht}_{ci}", tag="ps")
                        for kt in range(KD):
                            nc.tensor.matmul(
                                pu[:, :],
                                ut[:, kt, sub * P:(sub + 1) * P],
                                xts[ci][:, kt, :],
                                start=(kt == 0),
                                stop=(kt == KD - 1),
                            )
                        if parts != "a":
                            nc.vector.tensor_mul(
                                hs[ci][ht][:, :], tmps[sub][:, :], pu[:, :]
                            )

            # ---- stage B ----
            if parts in ("a", "b"):
                continue
            for dc in range(2):
                pos = {}
                for ci in range(len(cts)):
                    for m in range(CT // P):
                        pos[(ci, m)] = pspool.tile(
                            [P, 512], F32, name=f"po_{dc}_{ci}_{m}", tag="ps"
                        )
                ots = [
                    opool.tile([P, CT // P, 512], F32, name=f"o_{dc}_{ci}", tag="o")
                    for ci in range(len(cts))
                ]
                for hf in range(4):
                    if first_sb:
                        dt_ = dwpool.tile(
                            [P, KH // 4, 512], MMD, name=f"d_{dc}_{hf}", tag="dw"
                        )
                        nc.sync.dma_start(
                            dt_[:, :, :],
                            dw[dc, hf // 2][:, (hf % 2) * 4:(hf % 2) * 4 + 4, :],
                        )
                        dts[(dc, hf)] = dt_
                    else:
                        dt_ = dts[(dc, hf)]
                    for kb in range(KH // 4):
                        kh = hf * (KH // 4) + kb
                        for ci in range(len(cts)):
                            for m in range(CT // P):
                                nc.tensor.matmul(
                                    pos[(ci, m)][:, :],
                                    hs[ci][kh][:, m * P:(m + 1) * P],
                                    dt_[:, kb, :],
                                    start=(kh == 0),
                                    stop=(kh == KH - 1),
                                )
                for ci, ct in enumerate(cts):
                    for m in range(CT // P):
                        j = ct * (CT // P) + m
                        nc.scalar.mul(
                            ots[ci][:, m, :], pos[(ci, m)][:, :], p_sb[:, j:j + 1]
                        )
                        if store_q == "split":
                            q = nc.gpsimd if m % 2 == 0 else nc.scalar
                        else:
                            q = nc.gpsimd
                        q.dma_start(y[ct, dc][:, m, :], ots[ci][:, m, :])


def strip_pe_incs(nc):
    """Drop PE counting-sem increments that no wait ever targets (Tile's
    optimize_sems is disabled upstream; every matmul incs the PE sem, an
    EVT_SEM write costing ~26ns on the engine, 1536/rep). Keep exactly the
    increments whose cumulative count appears as some wait threshold (plus
    the final one) and renumber thresholds — each wait still becomes
    satisfied at the completion of the exact same matmul as before, so the
    transform is semantics-preserving (no rounding, no added delay, no new
    dependency cycles).
    """
    fn = nc.m.functions[0]
    insts = [inst for bb in fn.blocks for inst in bb.instructions]
    # Locate the PE counting sem: the one matmuls inc.
    pe_id = None
    for inst in insts:
        if isinstance(inst, mybir.InstMatmult) and inst.sync_info:
            for u in inst.sync_info.on_update:
                if u.sync_type == "semaphore" and u.update_mode == "sem-inc":
                    pe_id = u.id
                    break
        if pe_id is not None:
            break
    if pe_id is None:
        return
    # Enumerate inc events on that sem in program order.
    events = []  # instructions that sem-inc the PE sem, program order
    for inst in insts:
        si = inst.sync_info
        if not si:
            continue
        for u in si.on_update:
            if u.sync_type == "semaphore" and u.id == pe_id:
                if u.update_mode != "sem-inc" or u.update_value != 1:
                    continue  # e.g. the For_i reset's sem-set; leave as-is
                events.append(inst)
    if not events:
        return
    # Collect every wait threshold on the sem.
    targets = set()
    pe_waits = []
    for inst in insts:
        si = inst.sync_info
        if not si:
            continue
        for w in si.on_wait:
            if (
                w.sync_type == "semaphore"
                and w.id == pe_id
                and w.wait_mode == "sem-ge-imm"
                and w.wait_value > 0  # >=0 waits (skip path) are no-ops
            ):
                assert w.wait_value <= len(events)
                targets.add(w.wait_value)
                pe_waits.append(w)
    kept = sorted(targets | {len(events)})
    rank = {old: i + 1 for i, old in enumerate(kept)}
    for w in pe_waits:
        w.wait_value = rank[w.wait_value]
    # The For_i reset adjusts the sem by the old per-iteration total
    # (sem-add-imm/sem-sub-imm 1536) — rescale to the kept count.
    for inst in insts:
        si = inst.sync_info
        if not si:
            continue
        for u in si.on_update:
            if (
                u.sync_type == "semaphore"
                and u.id == pe_id
                and u.update_mode in ("sem-add-imm", "sem-sub-imm")
            ):
                assert u.update_value == len(events), (
                    f"unexpected bulk sem adjust {u.update_mode} "
                    f"{u.update_value} != {len(events)}"
                )
                u.update_value = len(kept)
    keep_set = set(kept)
    for i, inst in enumerate(events, 1):
        if i in keep_set:
            continue
        si = inst.sync_info
        new_upd = [
            u
            for u in si.on_update
            if not (u.sync_type == "semaphore" and u.id == pe_id)
        ]
        inst.sync_info = mybir.SyncInfo(
            on_wait=list(si.on_wait), on_update=new_upd
        )


def hoist_pe_waits(nc):
    """Move sem waits off InstLdweights/InstMatmult onto standalone PE
    EventSemaphore instructions inserted just before them. A wait-bearing
    engine instruction forces the PE NX out of hardware decode (~71ns SW
    decode + sem path vs 2.2ns); a seq-only wait keeps the engine stream
    HW-decoded. Ordering is identical: same engine, same program position.
    """
    fn = nc.m.functions[0]
    for bb in fn.blocks:
        # collect (index, inst) for wait-bearing PE engine instructions
        targets = []
        for i, inst in enumerate(bb.instructions):
            if isinstance(inst, (mybir.InstLdweights, mybir.InstMatmult)):
                si = inst.sync_info
                if si is not None and si.on_wait:
                    targets.append((i, inst))
        for i, inst in reversed(targets):
            si = inst.sync_info
            ev = mybir.InstEventSemaphore(
                name=nc.get_next_instruction_name(), ins=[], outs=[]
            )
            ev.engine = inst.engine
            ev.sync_info = mybir.SyncInfo(
                on_wait=list(si.on_wait), on_update=[]
            )
            inst.sync_info = mybir.SyncInfo(
                on_wait=[], on_update=list(si.on_update)
            )
            nc.register_instruction(ev)
            bb.instructions.insert(i, ev)


def build_nc(reps_loop=False, max_reps=512, mm=None, strip=True, ver=4, parts="full", store_q="split", hoist=False):
    mmd = MM_DTYPES[mm or MM][0]
    """Build the per-core Bass program. With reps_loop, the whole body runs
    inside a For_i whose trip count is read from an int32 input "reps"."""
    nc = bacc.Bacc(None, target_bir_lowering=False)
    with tile.TileContext(nc) as tc:
        xt = nc.dram_tensor("xt", [NCT, P, KD, CT], mmd, kind="ExternalInput")
        gw = nc.dram_tensor("gw", [NH4, P, KD, 512], mmd, kind="ExternalInput")
        uw = nc.dram_tensor("uw", [NH4, P, KD, 512], mmd, kind="ExternalInput")
        dw = nc.dram_tensor("dw", [2, 2, P, KH // 2, 512], mmd, kind="ExternalInput")
        pv = nc.dram_tensor("pv", [P, C // P], F32, kind="ExternalInput")
        y = nc.dram_tensor("y", [NCT, 2, P, CT // P, 512], F32, kind="ExternalOutput")
        if ver == 4:
            def emit(warm):
                emit_expert_ffn_v4(tc, xt, gw, uw, dw, pv, y, MMD=mmd, warm=warm, parts=parts, store_q=store_q)
        elif ver == 3:
            def emit(warm):
                emit_expert_ffn_v3(tc, xt, gw, uw, dw, pv, y, MMD=mmd, warm=warm)
        elif ver == 2:
            def emit(warm):
                emit_expert_ffn_v2(tc, xt, gw, uw, dw, pv, y, MMD=mmd, warm=warm)
        else:
            def emit(warm):
                emit_expert_ffn(tc, xt, gw, uw, dw, pv, y, MMD=mmd)
        if reps_loop:
            reps = nc.dram_tensor("reps", [1, 1], mybir.dt.int32, kind="ExternalInput")
            with tc.tile_pool(name="rpool", bufs=1) as rpool, \
                 tc.tile_pool(name="spool", bufs=1) as spool:
                r_sb = rpool.tile([1, 1], mybir.dt.int32)
                nc.sync.dma_start(r_sb[:, :], reps[:, :])
                rv = nc.values_load(
                    r_sb[0:1, 0:1],
                    min_val=0,
                    max_val=max_reps,
                    skip_runtime_bounds_check=True,
                )
                warm = None
                if ver >= 2:
                    warm = spool.tile([P, 512], mmd, name="warm")
                    nc.vector.memset(warm[:, :], 0.0)
                with tc.For_i(0, rv, 1):
                    emit(warm)
        else:
            with tc.tile_pool(name="spool", bufs=1) as spool:
                warm = None
                if ver >= 2:
                    warm = spool.tile([P, 512], mmd, name="warm")
                    nc.vector.memset(warm[:, :], 0.0)
                emit(warm)
    nc.compile()
    # strip AFTER compile: the compile passes (move_matmul_waits_to_ldweights,
    # generate_event_semaphores, loop lowering) re-derive sem totals, so
    # rewriting before them leaves stale counts behind
    if strip:
        strip_pe_incs(nc)
    if hoist:
        hoist_pe_waits(nc)
    return nc


def pack_inputs(x_pad, gate_w_e, up_w_e, down_w_e, p_pad, mm=None):
    npdt = MM_DTYPES[mm or MM][1]
    """Pack one expert's inputs into the SBUF-tile-order DRAM layouts.
    Matmul operands are cast to bf16 (cast first: halves the transpose
    bytes)."""
    # xt [NCT, 128, KD, 512]: [ct, p, kt, tok] = x_pad[ct*512+tok, kt*128+p]
    xt = np.ascontiguousarray(
        x_pad.astype(npdt).reshape(NCT, CT, KD, P).transpose(0, 3, 2, 1)
    )
    # gw/uw [NH4, 128, KD, 512]: [b, p, kt, h] = w[b*512+h, kt*128+p]
    gw = np.ascontiguousarray(
        gate_w_e.astype(npdt).reshape(NH4, 512, KD, P).transpose(0, 3, 2, 1)
    )
    uw = np.ascontiguousarray(
        up_w_e.astype(npdt).reshape(NH4, 512, KD, P).transpose(0, 3, 2, 1)
    )
    # dw [2, 2, 128, KH//2, 512]: [dc, hf, p, kb, d] = down[dc*512+d, hf*1024+kb*128+p]
    dw = np.ascontiguousarray(
        down_w_e.astype(npdt).reshape(2, 512, 2, KH // 2, P).transpose(0, 2, 4, 3, 1)
    )
    pv = np.ascontiguousarray(p_pad.reshape(C // P, P).T)
    return {"xt": xt, "gw": gw, "uw": uw, "dw": dw, "pv": pv}


def unpack_y(y_pack):
    """y_pack [NCT, 2, 128, 4, 512] -> y [C, D]."""
    return np.ascontiguousarray(
        y_pack.transpose(0, 3, 2, 1, 4).reshape(C, D)
    )


def route_and_dispatch(x, router_w):
    """Host router + top-2 dispatch (matches softmax/top_k/renorm of the
    reference exactly)."""
    logits = x @ router_w.T                      # [T, E]
    t_ar = np.arange(T)
    i1 = np.argmax(logits, axis=1)
    l1 = logits[t_ar, i1]
    lm = logits.copy()
    lm[t_ar, i1] = -np.inf
    i2 = np.argmax(lm, axis=1)
    l2 = lm[t_ar, i2]
    e2 = np.exp(l2 - l1)
    p1 = 1.0 / (1.0 + e2)
    p2 = e2 / (1.0 + e2)

    ee = np.concatenate([i1, i2])                # [2T] expert of each pair
    tt = np.concatenate([t_ar, t_ar])            # [2T] token of each pair
    pp = np.concatenate([p1, p2]).astype(np.float32)
    counts = np.bincount(ee, minlength=E)
    starts = np.zeros(E, np.int64)
    starts[1:] = np.cumsum(counts)[:-1]
    order = np.argsort(ee, kind="stable")
    pos = np.empty(2 * T, np.int64)
    pos[order] = np.arange(2 * T) - starts[ee[order]]
    return ee, tt, pp, pos, counts, starts, order


def kernel(**inputs):
    x = np.ascontiguousarray(
        np.asarray(inputs["hidden_states"], np.float32).reshape(T, D)
    )
    router_w = np.asarray(inputs["router_w"], np.float32)
    gate_w = np.asarray(inputs["gate_w"], np.float32)
    up_w = np.asarray(inputs["up_w"], np.float32)
    down_w = np.asarray(inputs["down_w"], np.float32)

    ee, tt, pp, pos, counts, starts, order = route_and_dispatch(x, router_w)

    in_maps = []
    for e in range(E):
        n_e = min(int(counts[e]), C)
        sel = order[starts[e]:starts[e] + n_e]   # pairs dispatched to core e
        xp = np.zeros((C, D), np.float32)
        xp[:n_e] = x[tt[sel]]
        pvec = np.zeros(C, np.float32)
        pvec[:n_e] = pp[sel]
        in_maps.append(pack_inputs(xp, gate_w[e], up_w[e], down_w[e], pvec))

    nc = build_nc()
    res = run_bass_kernel_spmd(nc, in_maps, core_ids=list(range(E)))
    ys = np.stack(
        [unpack_y(res.results[e]["y"]) for e in range(E)]
    ).reshape(E * C, D)

    ok = pos < C
    contrib = np.zeros((2 * T, D), np.float32)
    g = ee * C + pos
    contrib[ok] = ys[g[ok]]
    # capacity-overflow fallback: exact fp32 host compute for the few pairs
    # beyond capacity (~0.8% of pairs for the seed-0 routing), batched per
    # expert
    if not ok.all():
        bad = np.nonzero(~ok)[0]
        for e in np.unique(ee[bad]):
            js = bad[ee[bad] == e]
            xb = x[tt[js]]
            gb = xb @ gate_w[e].T
            ub = xb @ up_w[e].T
            hb = (gb / (1.0 + np.exp(-gb))) * ub
            contrib[js] = (hb @ down_w[e].T) * pp[js, None]
    out = contrib[:T] + contrib[T:]
    return out.reshape(B, S, D).astype(np.float32)



# revision 29
# speedup vs baseline: 1.2179x; 1.0412x over previous
"""MoE layer (top-2 of 8 experts) for 8 Trainium2 NeuronCores.

Strategy: expert-parallel. Host computes the (tiny) router + top-2 dispatch in
numpy; core e runs expert e's FFN over its dispatched tokens (padded to a fixed
capacity C) with bf16 matmuls (fp32 PSUM accumulation); host combines the two
expert outputs per token.

All device matmuls are [K=128]x[M=128]x[N=512] bf16 (1 cycle/row):
  gate^T/up^T [H, Ct] = gwT/uwT.T @ xt   (contraction over D, 8 k-tiles)
  h = silu(gate) * up                    (SBUF-resident [128, 512] tiles)
  y [Ct, D] = (h.T @ dwT) * p            (contraction over H, 16 k-tiles,
                                          combine-prob scale fused in eviction)

The host pre-packs weights and tokens into SBUF-tile order so every load is a
large DMA with contiguous per-partition lines. v4 schedule: weights are
SBUF-resident (loaded once per invocation at just-in-time program positions),
tokens all hoisted to the start, PE warmup matmuls on a zeroed scratch tile
bridge the initial DMA wait, the down-projection runs K-contiguous per output
group (16 accumulating matmuls back-to-back, immediate eviction + store, fast
PSUM bank recycling), and redundant per-matmul PE semaphore increments are
stripped post-compile (strip_pe_incs).
"""

import ml_dtypes
import numpy as np

import concourse.bass as bass
import concourse.mybir as mybir
import concourse.tile as tile
from concourse import bacc
from concourse.bass_utils import run_bass_kernel_spmd

E = 8
TOP_K = 2
B, S, D, H = 4, 2048, 1024, 2048
T = B * S
C = 2048          # per-expert token capacity; overflow pairs (seed-0: ~137
                  # of 16384, counts max 2175) fall back to exact host compute
CT = 512          # token tile
P = 128
NCT = C // CT     # 4
KD = D // P       # 8  k-tiles for gate/up
KH = H // P       # 16 k-tiles for down
NH4 = H // 512    # 4  groups of 4 h-blocks
F32 = mybir.dt.float32
F32R = mybir.dt.float32r
# matmul operand dtype: bf16 streams at the same 1 cycle/row as fp32r, but
# enables FWL + LDWEIGHTS pull-ahead (fp32 weight loads serialize inside the
# self-loading matmul) and halves DMA traffic. PSUM accumulation stays fp32.
BF16 = mybir.dt.bfloat16
NP_BF16 = ml_dtypes.bfloat16
MM_DTYPES = {"bf16": (BF16, NP_BF16), "f32r": (F32R, np.float32)}
MM = "bf16"   # default matmul-operand dtype
AF = mybir.ActivationFunctionType


def emit_expert_ffn(tc, xt, gw, uw, dw, pv, y, MMD=None):
    MMD = MMD or MM_DTYPES[MM][0]
    """Emit one expert's FFN.

    DRAM tensors (all pre-packed on host):
      xt [NCT, 128, KD, 512] f32r - tokens, transposed per ct tile
      gw/uw [NH4, 128, KD, 512] f32r - gate/up weights per 4-h-block group
      dw [2, 2, 128, KH//2, 512] f32r - down weights per (dc, kh-half)
      pv [128, C//128] f32 - combine probs (token-partition layout)
      y  [NCT, 2, 128, 4, 512] f32 out - [ct, dc, p, m, 512]
    """
    nc = tc.nc
    # superblocks of up to 2 token tiles sharing one weight pass
    sbs = [list(range(s, min(s + 2, NCT))) for s in range(0, NCT, 2)]

    with (
        tc.tile_pool(name="xpool", bufs=2) as xpool,
        tc.tile_pool(name="wpool", bufs=3) as wpool,
        tc.tile_pool(name="hpool", bufs=36) as hpool,
        tc.tile_pool(name="dpool", bufs=3) as dpool,
        tc.tile_pool(name="tpool", bufs=5) as tpool,
        tc.tile_pool(name="opool", bufs=2) as opool,
        tc.tile_pool(name="ppool", bufs=1) as ppool,
        tc.tile_pool(name="pspool", bufs=8, space="PSUM") as pspool,
    ):
        p_sb = ppool.tile([P, C // P], F32)
        nc.gpsimd.dma_start(p_sb[:, :], pv[:, :])

        for cts in sbs:
            # ---- token tiles: one 2MB DMA per ct ----
            xts = []
            for ct in cts:
                x_t = xpool.tile([P, KD, CT], MMD, name=f"xt_{ct}", tag="xt")
                nc.gpsimd.dma_start(x_t[:, 0:4, :], xt[ct][:, 0:4, :])
                nc.gpsimd.dma_start(x_t[:, 4:8, :], xt[ct][:, 4:8, :])
                xts.append(x_t)
            hs = [
                [
                    hpool.tile([P, CT], MMD, name=f"h_{ct}_{ht}", tag="h")
                    for ht in range(KH)
                ]
                for ct in cts
            ]

            # ---- stage A: gate/up matmuls + silu*mul -> h ----
            first_sb = cts[0] == 0
            for ht4 in range(NH4):
                gt = wpool.tile([P, KD, 512], MMD, name=f"g_{ht4}", tag="w")
                if ht4 == 0 and first_sb:
                    # quarter-granularity on the very first load so the first
                    # matmuls start ~2us earlier out of the cold start
                    for q in range(4):
                        nc.sync.dma_start(
                            gt[:, 2 * q:2 * q + 2, :], gw[ht4][:, 2 * q:2 * q + 2, :]
                        )
                else:
                    nc.sync.dma_start(gt[:, 0:4, :], gw[ht4][:, 0:4, :])
                    nc.sync.dma_start(gt[:, 4:8, :], gw[ht4][:, 4:8, :])
                ut = wpool.tile([P, KD, 512], MMD, name=f"u_{ht4}", tag="w")
                nc.scalar.dma_start(ut[:, 0:4, :], uw[ht4][:, 0:4, :])
                nc.scalar.dma_start(ut[:, 4:8, :], uw[ht4][:, 4:8, :])
                # ct-major, all-gate-then-all-up: gt's last use lands at ~75%
                # of the group so the next group's weight DMA overlaps compute
                for ci in range(len(cts)):
                    tmps = []
                    for sub in range(4):
                        ht = ht4 * 4 + sub
                        pg = pspool.tile([P, CT], F32, name=f"pg_{ht}_{ci}", tag="ps")
                        for kt in range(KD):
                            nc.tensor.matmul(
                                pg[:, :],
                                gt[:, kt, sub * P:(sub + 1) * P],
                                xts[ci][:, kt, :],
                                start=(kt == 0),
                                stop=(kt == KD - 1),
                            )
                        tmp = tpool.tile([P, CT], MMD, name=f"t_{ht}_{ci}", tag="t")
                        nc.scalar.activation(tmp[:, :], pg[:, :], AF.Silu)
                        tmps.append(tmp)
                    for sub in range(4):
                        ht = ht4 * 4 + sub
                        pu = pspool.tile([P, CT], F32, name=f"pu_{ht}_{ci}", tag="ps")
                        for kt in range(KD):
                            nc.tensor.matmul(
                                pu[:, :],
                                ut[:, kt, sub * P:(sub + 1) * P],
                                xts[ci][:, kt, :],
                                start=(kt == 0),
                                stop=(kt == KD - 1),
                            )
                        nc.vector.tensor_mul(
                            hs[ci][ht][:, :], tmps[sub][:, :], pu[:, :]
                        )

            # ---- stage B: down matmuls + prob scale -> y ----
            for dc in range(2):
                pos = {}
                for ci in range(len(cts)):
                    for m in range(CT // P):
                        pos[(ci, m)] = pspool.tile(
                            [P, 512], F32, name=f"po_{dc}_{ci}_{m}", tag="ps"
                        )
                ots = [
                    opool.tile([P, CT // P, 512], F32, name=f"o_{dc}_{ci}", tag="o")
                    for ci in range(len(cts))
                ]
                for hf in range(4):
                    dt_ = dpool.tile([P, KH // 4, 512], MMD, name=f"d_{dc}_{hf}", tag="dw")
                    nc.sync.dma_start(
                        dt_[:, :, :], dw[dc, hf // 2][:, (hf % 2) * 4:(hf % 2) * 4 + 4, :]
                    )
                    for kb in range(KH // 4):
                        kh = hf * (KH // 4) + kb
                        for ci in range(len(cts)):
                            for m in range(CT // P):
                                nc.tensor.matmul(
                                    pos[(ci, m)][:, :],
                                    hs[ci][kh][:, m * P:(m + 1) * P],
                                    dt_[:, kb, :],
                                    start=(kh == 0),
                                    stop=(kh == KH - 1),
                                )
                for ci, ct in enumerate(cts):
                    for m in range(CT // P):
                        j = ct * (CT // P) + m
                        nc.scalar.mul(
                            ots[ci][:, m, :], pos[(ci, m)][:, :], p_sb[:, j:j + 1]
                        )
                        # per-m stores start as soon as each eviction lands,
                        # shortening the kernel-tail drain
                        nc.gpsimd.dma_start(y[ct, dc][:, m, :], ots[ci][:, m, :])


def emit_expert_ffn_v2(tc, xt, gw, uw, dw, pv, y, MMD=None, warm=None):
    """v2: all xt loads hoisted to rep start on the vector DMA queue (first
    ct at quarter granularity for a fast cold start), PE warmup matmuls on a
    zero scratch tile to bridge the head DMA wait (keeps HAM at K=8/8 across
    For_i reps), y stores split across the gpsimd and vector queues."""
    MMD = MMD or MM_DTYPES[MM][0]
    nc = tc.nc
    sbs = [list(range(s, min(s + 2, NCT))) for s in range(0, NCT, 2)]

    with (
        tc.tile_pool(name="xpool", bufs=4) as xpool,
        tc.tile_pool(name="wpool", bufs=3) as wpool,
        tc.tile_pool(name="hpool", bufs=36) as hpool,
        tc.tile_pool(name="dpool", bufs=3) as dpool,
        tc.tile_pool(name="tpool", bufs=5) as tpool,
        tc.tile_pool(name="opool", bufs=2) as opool,
        tc.tile_pool(name="ppool", bufs=1) as ppool,
        tc.tile_pool(name="pspool", bufs=8, space="PSUM") as pspool,
    ):
        p_sb = ppool.tile([P, C // P], F32)
        nc.gpsimd.dma_start(p_sb[:, :], pv[:, :])

        # all token tiles up front on the vector queue
        xts_all = []
        for ct in range(NCT):
            x_t = xpool.tile([P, KD, CT], MMD, name=f"xt_{ct}", tag="xt")
            if ct == 0:
                for q in range(4):
                    nc.gpsimd.dma_start(
                        x_t[:, 2 * q:2 * q + 2, :], xt[ct][:, 2 * q:2 * q + 2, :]
                    )
            else:
                nc.gpsimd.dma_start(x_t[:, 0:4, :], xt[ct][:, 0:4, :])
                nc.gpsimd.dma_start(x_t[:, 4:8, :], xt[ct][:, 4:8, :])
            xts_all.append(x_t)

        # PE warmup: dummy matmuls on the preamble-zeroed scratch tile; no
        # DMA dependencies, so the PE chews these while the first loads land
        if warm is not None:
            wps = pspool.tile([P, 512], F32, name="warm_ps", tag="ps")
            for _ in range(12):
                nc.tensor.matmul(
                    wps[:, :], warm[:, 0:P], warm[:, :], start=True, stop=True
                )

        for cts in sbs:
            xts = [xts_all[ct] for ct in cts]
            hs = [
                [
                    hpool.tile([P, CT], MMD, name=f"h_{ct}_{ht}", tag="h")
                    for ht in range(KH)
                ]
                for ct in cts
            ]

            # ---- stage A: gate/up matmuls + silu*mul -> h ----
            first_sb = cts[0] == 0
            for ht4 in range(NH4):
                gt = wpool.tile([P, KD, 512], MMD, name=f"g_{ht4}", tag="w")
                if ht4 == 0 and first_sb:
                    for q in range(4):
                        nc.sync.dma_start(
                            gt[:, 2 * q:2 * q + 2, :], gw[ht4][:, 2 * q:2 * q + 2, :]
                        )
                else:
                    nc.sync.dma_start(gt[:, 0:4, :], gw[ht4][:, 0:4, :])
                    nc.sync.dma_start(gt[:, 4:8, :], gw[ht4][:, 4:8, :])
                ut = wpool.tile([P, KD, 512], MMD, name=f"u_{ht4}", tag="w")
                nc.scalar.dma_start(ut[:, 0:4, :], uw[ht4][:, 0:4, :])
                nc.scalar.dma_start(ut[:, 4:8, :], uw[ht4][:, 4:8, :])
                for ci in range(len(cts)):
                    tmps = []
                    for sub in range(4):
                        ht = ht4 * 4 + sub
                        pg = pspool.tile([P, CT], F32, name=f"pg_{ht}_{ci}", tag="ps")
                        for kt in range(KD):
                            nc.tensor.matmul(
                                pg[:, :],
                                gt[:, kt, sub * P:(sub + 1) * P],
                                xts[ci][:, kt, :],
                                start=(kt == 0),
                                stop=(kt == KD - 1),
                            )
                        tmp = tpool.tile([P, CT], MMD, name=f"t_{ht}_{ci}", tag="t")
                        nc.scalar.activation(tmp[:, :], pg[:, :], AF.Silu)
                        tmps.append(tmp)
                    for sub in range(4):
                        ht = ht4 * 4 + sub
                        pu = pspool.tile([P, CT], F32, name=f"pu_{ht}_{ci}", tag="ps")
                        for kt in range(KD):
                            nc.tensor.matmul(
                                pu[:, :],
                                ut[:, kt, sub * P:(sub + 1) * P],
                                xts[ci][:, kt, :],
                                start=(kt == 0),
                                stop=(kt == KD - 1),
                            )
                        nc.vector.tensor_mul(
                            hs[ci][ht][:, :], tmps[sub][:, :], pu[:, :]
                        )

            # ---- stage B: down matmuls + prob scale -> y ----
            for dc in range(2):
                pos = {}
                for ci in range(len(cts)):
                    for m in range(CT // P):
                        pos[(ci, m)] = pspool.tile(
                            [P, 512], F32, name=f"po_{dc}_{ci}_{m}", tag="ps"
                        )
                ots = [
                    opool.tile([P, CT // P, 512], F32, name=f"o_{dc}_{ci}", tag="o")
                    for ci in range(len(cts))
                ]
                for hf in range(4):
                    dt_ = dpool.tile([P, KH // 4, 512], MMD, name=f"d_{dc}_{hf}", tag="dw")
                    nc.sync.dma_start(
                        dt_[:, :, :], dw[dc, hf // 2][:, (hf % 2) * 4:(hf % 2) * 4 + 4, :]
                    )
                    for kb in range(KH // 4):
                        kh = hf * (KH // 4) + kb
                        for ci in range(len(cts)):
                            for m in range(CT // P):
                                nc.tensor.matmul(
                                    pos[(ci, m)][:, :],
                                    hs[ci][kh][:, m * P:(m + 1) * P],
                                    dt_[:, kb, :],
                                    start=(kh == 0),
                                    stop=(kh == KH - 1),
                                )
                for ci, ct in enumerate(cts):
                    for m in range(CT // P):
                        j = ct * (CT // P) + m
                        nc.scalar.mul(
                            ots[ci][:, m, :], pos[(ci, m)][:, :], p_sb[:, j:j + 1]
                        )
                        q = nc.gpsimd if m % 2 == 0 else nc.scalar
                        q.dma_start(y[ct, dc][:, m, :], ots[ci][:, m, :])


def emit_expert_ffn_v3(tc, xt, gw, uw, dw, pv, y, MMD=None, warm=None):
    """v3: fully SBUF-resident weights/tokens (bf16 fits: ~186KB/partition).
    All input DMAs issue up front, ordered by first-use time across the three
    queues, so no matmul ever waits on a mid-rep load; the second superblock
    runs with zero input DMAs. Warmup matmuls bridge the initial load."""
    MMD = MMD or MM_DTYPES[MM][0]
    nc = tc.nc
    sbs = [list(range(s, min(s + 2, NCT))) for s in range(0, NCT, 2)]

    with (
        tc.tile_pool(name="xpool", bufs=NCT) as xpool,
        tc.tile_pool(name="gpool", bufs=NH4) as gpool,
        tc.tile_pool(name="upool", bufs=NH4) as upool,
        tc.tile_pool(name="dwpool", bufs=8) as dwpool,
        tc.tile_pool(name="hpool", bufs=36) as hpool,
        tc.tile_pool(name="tpool", bufs=5) as tpool,
        tc.tile_pool(name="opool", bufs=2) as opool,
        tc.tile_pool(name="ppool", bufs=1) as ppool,
        tc.tile_pool(name="pspool", bufs=8, space="PSUM") as pspool,
    ):
        p_sb = ppool.tile([P, C // P], F32)
        nc.gpsimd.dma_start(p_sb[:, :], pv[:, :])

        # ---- all input loads up front, ordered by first use ----
        # gpsimd queue: tokens (ct0 at quarter granularity)
        xts_all = []
        for ct in range(NCT):
            x_t = xpool.tile([P, KD, CT], MMD, name=f"xt_{ct}", tag="xt")
            if ct == 0:
                for q in range(4):
                    nc.gpsimd.dma_start(
                        x_t[:, 2 * q:2 * q + 2, :], xt[ct][:, 2 * q:2 * q + 2, :]
                    )
            else:
                nc.gpsimd.dma_start(x_t[:, 0:4, :], xt[ct][:, 0:4, :])
                nc.gpsimd.dma_start(x_t[:, 4:8, :], xt[ct][:, 4:8, :])
            xts_all.append(x_t)
        # sync queue: gate weights then down dc1; scalar: up weights then dc0
        gts, uts = [], []
        for g in range(NH4):
            gt = gpool.tile([P, KD, 512], MMD, name=f"g_{g}", tag="gw")
            if g == 0:
                for q in range(4):
                    nc.sync.dma_start(
                        gt[:, 2 * q:2 * q + 2, :], gw[g][:, 2 * q:2 * q + 2, :]
                    )
            else:
                nc.sync.dma_start(gt[:, 0:4, :], gw[g][:, 0:4, :])
                nc.sync.dma_start(gt[:, 4:8, :], gw[g][:, 4:8, :])
            gts.append(gt)
        for g in range(NH4):
            ut = upool.tile([P, KD, 512], MMD, name=f"u_{g}", tag="uw")
            nc.scalar.dma_start(ut[:, 0:4, :], uw[g][:, 0:4, :])
            nc.scalar.dma_start(ut[:, 4:8, :], uw[g][:, 4:8, :])
            uts.append(ut)
        dts = {}
        for dc in range(2):
            for hf in range(4):
                dt_ = dwpool.tile(
                    [P, KH // 4, 512], MMD, name=f"d_{dc}_{hf}", tag="dw"
                )
                q = nc.scalar if dc == 0 else nc.sync
                q.dma_start(
                    dt_[:, :, :],
                    dw[dc, hf // 2][:, (hf % 2) * 4:(hf % 2) * 4 + 4, :],
                )
                dts[(dc, hf)] = dt_

        # PE warmup on the preamble-zeroed scratch: no DMA deps
        if warm is not None:
            wps = pspool.tile([P, 512], F32, name="warm_ps", tag="ps")
            for _ in range(12):
                nc.tensor.matmul(
                    wps[:, :], warm[:, 0:P], warm[:, :], start=True, stop=True
                )

        for cts in sbs:
            xts = [xts_all[ct] for ct in cts]
            hs = [
                [
                    hpool.tile([P, CT], MMD, name=f"h_{ct}_{ht}", tag="h")
                    for ht in range(KH)
                ]
                for ct in cts
            ]

            # ---- stage A: gate/up matmuls + silu*mul -> h ----
            for ht4 in range(NH4):
                gt, ut = gts[ht4], uts[ht4]
                for ci in range(len(cts)):
                    tmps = []
                    for sub in range(4):
                        ht = ht4 * 4 + sub
                        pg = pspool.tile([P, CT], F32, name=f"pg_{ht}_{ci}", tag="ps")
                        for kt in range(KD):
                            nc.tensor.matmul(
                                pg[:, :],
                                gt[:, kt, sub * P:(sub + 1) * P],
                                xts[ci][:, kt, :],
                                start=(kt == 0),
                                stop=(kt == KD - 1),
                            )
                        tmp = tpool.tile([P, CT], MMD, name=f"t_{ht}_{ci}", tag="t")
                        nc.scalar.activation(tmp[:, :], pg[:, :], AF.Silu)
                        tmps.append(tmp)
                    for sub in range(4):
                        ht = ht4 * 4 + sub
                        pu = pspool.tile([P, CT], F32, name=f"pu_{ht}_{ci}", tag="ps")
                        for kt in range(KD):
                            nc.tensor.matmul(
                                pu[:, :],
                                ut[:, kt, sub * P:(sub + 1) * P],
                                xts[ci][:, kt, :],
                                start=(kt == 0),
                                stop=(kt == KD - 1),
                            )
                        nc.vector.tensor_mul(
                            hs[ci][ht][:, :], tmps[sub][:, :], pu[:, :]
                        )

            # ---- stage B: down matmuls + prob scale -> y ----
            for dc in range(2):
                pos = {}
                for ci in range(len(cts)):
                    for m in range(CT // P):
                        pos[(ci, m)] = pspool.tile(
                            [P, 512], F32, name=f"po_{dc}_{ci}_{m}", tag="ps"
                        )
                ots = [
                    opool.tile([P, CT // P, 512], F32, name=f"o_{dc}_{ci}", tag="o")
                    for ci in range(len(cts))
                ]
                for hf in range(4):
                    dt_ = dts[(dc, hf)]
                    for kb in range(KH // 4):
                        kh = hf * (KH // 4) + kb
                        for ci in range(len(cts)):
                            for m in range(CT // P):
                                nc.tensor.matmul(
                                    pos[(ci, m)][:, :],
                                    hs[ci][kh][:, m * P:(m + 1) * P],
                                    dt_[:, kb, :],
                                    start=(kh == 0),
                                    stop=(kh == KH - 1),
                                )
                for ci, ct in enumerate(cts):
                    for m in range(CT // P):
                        j = ct * (CT // P) + m
                        nc.scalar.mul(
                            ots[ci][:, m, :], pos[(ci, m)][:, :], p_sb[:, j:j + 1]
                        )
                        q = nc.gpsimd if m % 2 == 0 else nc.scalar
                        q.dma_start(y[ct, dc][:, m, :], ots[ci][:, m, :])


def emit_expert_ffn_v4(tc, xt, gw, uw, dw, pv, y, MMD=None, warm=None, parts="full", store_q="split", kcontig=False):
    """v4: v2's just-in-time DMA schedule + v3's weight residency. Weights
    load once per rep at the same program positions as v2 (spread issue, no
    bandwidth burst) into static tiles; superblock 1 then runs with zero
    input DMAs. Tokens all hoisted on gpsimd; warmup matmuls bridge the
    head; y stores split across gpsimd/scalar queues."""
    MMD = MMD or MM_DTYPES[MM][0]
    nc = tc.nc
    sbs = [list(range(s, min(s + 2, NCT))) for s in range(0, NCT, 2)]

    with (
        tc.tile_pool(name="xpool", bufs=NCT) as xpool,
        tc.tile_pool(name="gpool", bufs=NH4) as gpool,
        tc.tile_pool(name="upool", bufs=NH4) as upool,
        tc.tile_pool(name="dwpool", bufs=8) as dwpool,
        tc.tile_pool(name="hpool", bufs=36) as hpool,
        tc.tile_pool(name="tpool", bufs=5) as tpool,
        tc.tile_pool(name="opool", bufs=2) as opool,
        tc.tile_pool(name="ppool", bufs=1) as ppool,
        tc.tile_pool(name="pspool", bufs=8, space="PSUM") as pspool,
    ):
        p_sb = ppool.tile([P, C // P], F32)
        nc.gpsimd.dma_start(p_sb[:, :], pv[:, :])

        # tokens up front on gpsimd (ct0 at quarter granularity)
        xts_all = []
        for ct in range(NCT):
            x_t = xpool.tile([P, KD, CT], MMD, name=f"xt_{ct}", tag="xt")
            if ct == 0:
                for q in range(4):
                    nc.gpsimd.dma_start(
                        x_t[:, 2 * q:2 * q + 2, :], xt[ct][:, 2 * q:2 * q + 2, :]
                    )
            else:
                nc.gpsimd.dma_start(x_t[:, 0:4, :], xt[ct][:, 0:4, :])
                nc.gpsimd.dma_start(x_t[:, 4:8, :], xt[ct][:, 4:8, :])
            xts_all.append(x_t)

        if warm is not None:
            wps = pspool.tile([P, 512], F32, name="warm_ps", tag="ps")
            for _ in range(12):
                nc.tensor.matmul(
                    wps[:, :], warm[:, 0:P], warm[:, :], start=True, stop=True
                )

        gts, uts, dts = [None] * NH4, [None] * NH4, {}

        for cts in sbs:
            first_sb = cts[0] == 0
            xts = [xts_all[ct] for ct in cts]
            hs = [
                [
                    hpool.tile([P, CT], MMD, name=f"h_{ct}_{ht}", tag="h")
                    for ht in range(KH)
                ]
                for ct in cts
            ]

            # ---- stage A ----
            for ht4 in range(NH4):
                if first_sb:
                    gt = gpool.tile([P, KD, 512], MMD, name=f"g_{ht4}", tag="gw")
                    if ht4 == 0:
                        for q in range(4):
                            nc.sync.dma_start(
                                gt[:, 2 * q:2 * q + 2, :],
                                gw[ht4][:, 2 * q:2 * q + 2, :],
                            )
                    else:
                        nc.sync.dma_start(gt[:, 0:4, :], gw[ht4][:, 0:4, :])
                        nc.sync.dma_start(gt[:, 4:8, :], gw[ht4][:, 4:8, :])
                    ut = upool.tile([P, KD, 512], MMD, name=f"u_{ht4}", tag="uw")
                    nc.scalar.dma_start(ut[:, 0:4, :], uw[ht4][:, 0:4, :])
                    nc.scalar.dma_start(ut[:, 4:8, :], uw[ht4][:, 4:8, :])
                    gts[ht4], uts[ht4] = gt, ut
                else:
                    gt, ut = gts[ht4], uts[ht4]
                for ci in range(len(cts)):
                    tmps = []
                    for sub in range(4):
                        ht = ht4 * 4 + sub
                        pg = pspool.tile([P, CT], F32, name=f"pg_{ht}_{ci}", tag="ps")
                        for kt in range(KD):
                            nc.tensor.matmul(
                                pg[:, :],
                                gt[:, kt, sub * P:(sub + 1) * P],
                                xts[ci][:, kt, :],
                                start=(kt == 0),
                                stop=(kt == KD - 1),
                            )
                        if parts != "a":
                            tmp = tpool.tile([P, CT], MMD, name=f"t_{ht}_{ci}", tag="t")
                            nc.scalar.activation(tmp[:, :], pg[:, :], AF.Silu)
                            tmps.append(tmp)
                    for sub in range(4):
                        ht = ht4 * 4 + sub
                        pu = pspool.tile([P, CT], F32, name=f"pu_{ht}_{ci}", tag="ps")
                        for kt in range(KD):
                            nc.tensor.matmul(
                                pu[:, :],
                                ut[:, kt, sub * P:(sub + 1) * P],
                                xts[ci][:, kt, :],
                                start=(kt == 0),
                                stop=(kt == KD - 1),
                            )
                        if parts != "a":
                            nc.vector.tensor_mul(
                                hs[ci][ht][:, :], tmps[sub][:, :], pu[:, :]
                            )

            # ---- stage B ----
            if parts in ("a", "b"):
                continue
            for dc in range(2):
                pos = {}
                for ci in range(len(cts)):
                    for m in range(CT // P):
                        pos[(ci, m)] = pspool.tile(
                            [P, 512], F32, name=f"po_{dc}_{ci}_{m}", tag="ps"
                        )
                ots = [
                    opool.tile([P, CT // P, 512], F32, name=f"o_{dc}_{ci}", tag="o")
                    for ci in range(len(cts))
                ]
                if first_sb:
                    for hf in range(4):
                        dt_ = dwpool.tile(
                            [P, KH // 4, 512], MMD, name=f"d_{dc}_{hf}", tag="dw"
                        )
                        nc.sync.dma_start(
                            dt_[:, :, :],
                            dw[dc, hf // 2][:, (hf % 2) * 4:(hf % 2) * 4 + 4, :],
                        )
                        dts[(dc, hf)] = dt_
                if kcontig:
                    # K-contiguous: each (ci,m) group's 16 accumulating MMs run
                    # back-to-back, evict + store immediately, freeing the
                    # PSUM bank for the next group (dw tiles are resident)
                    for ci, ct in enumerate(cts):
                        for m in range(CT // P):
                            po = pos[(ci, m)]
                            for kh in range(KH):
                                nc.tensor.matmul(
                                    po[:, :],
                                    hs[ci][kh][:, m * P:(m + 1) * P],
                                    dts[(dc, kh // 4)][:, kh % 4, :],
                                    start=(kh == 0),
                                    stop=(kh == KH - 1),
                                )
                            j = ct * (CT // P) + m
                            nc.scalar.mul(
                                ots[ci][:, m, :], po[:, :], p_sb[:, j:j + 1]
                            )
                            if store_q == "split":
                                q = nc.gpsimd if m % 2 == 0 else nc.scalar
                            else:
                                q = nc.gpsimd
                            q.dma_start(y[ct, dc][:, m, :], ots[ci][:, m, :])
                else:
                    for hf in range(4):
                        dt_ = dts[(dc, hf)]
                        for kb in range(KH // 4):
                            kh = hf * (KH // 4) + kb
                            for ci in range(len(cts)):
                                for m in range(CT // P):
                                    nc.tensor.matmul(
                                        pos[(ci, m)][:, :],
                                        hs[ci][kh][:, m * P:(m + 1) * P],
                                        dt_[:, kb, :],
                                        start=(kh == 0),
                                        stop=(kh == KH - 1),
                                    )
                    for ci, ct in enumerate(cts):
                        for m in range(CT // P):
                            j = ct * (CT // P) + m
                            nc.scalar.mul(
                                ots[ci][:, m, :], pos[(ci, m)][:, :], p_sb[:, j:j + 1]
                            )
                            if store_q == "split":
                                q = nc.gpsimd if m % 2 == 0 else nc.scalar
                            else:
                                q = nc.gpsimd
                            q.dma_start(y[ct, dc][:, m, :], ots[ci][:, m, :])


def strip_pe_incs(nc):
    """Drop PE counting-sem increments that no wait ever targets (Tile's
    optimize_sems is disabled upstream; every matmul incs the PE sem, an
    EVT_SEM write costing ~26ns on the engine, 1536/rep). Keep exactly the
    increments whose cumulative count appears as some wait threshold (plus
    the final one) and renumber thresholds — each wait still becomes
    satisfied at the completion of the exact same matmul as before, so the
    transform is semantics-preserving (no rounding, no added delay, no new
    dependency cycles).
    """
    fn = nc.m.functions[0]
    insts = [inst for bb in fn.blocks for inst in bb.instructions]
    # Locate the PE counting sem: the one matmuls inc.
    pe_id = None
    for inst in insts:
        if isinstance(inst, mybir.InstMatmult) and inst.sync_info:
            for u in inst.sync_info.on_update:
                if u.sync_type == "semaphore" and u.update_mode == "sem-inc":
                    pe_id = u.id
                    break
        if pe_id is not None:
            break
    if pe_id is None:
        return
    # Enumerate inc events on that sem in program order.
    events = []  # instructions that sem-inc the PE sem, program order
    for inst in insts:
        si = inst.sync_info
        if not si:
            continue
        for u in si.on_update:
            if u.sync_type == "semaphore" and u.id == pe_id:
                if u.update_mode != "sem-inc" or u.update_value != 1:
                    continue  # e.g. the For_i reset's sem-set; leave as-is
                events.append(inst)
    if not events:
        return
    # Collect every wait threshold on the sem.
    targets = set()
    pe_waits = []
    for inst in insts:
        si = inst.sync_info
        if not si:
            continue
        for w in si.on_wait:
            if (
                w.sync_type == "semaphore"
                and w.id == pe_id
                and w.wait_mode == "sem-ge-imm"
                and w.wait_value > 0  # >=0 waits (skip path) are no-ops
            ):
                assert w.wait_value <= len(events)
                targets.add(w.wait_value)
                pe_waits.append(w)
    kept = sorted(targets | {len(events)})
    rank = {old: i + 1 for i, old in enumerate(kept)}
    for w in pe_waits:
        w.wait_value = rank[w.wait_value]
    # The For_i reset adjusts the sem by the old per-iteration total
    # (sem-add-imm/sem-sub-imm 1536) — rescale to the kept count.
    for inst in insts:
        si = inst.sync_info
        if not si:
            continue
        for u in si.on_update:
            if (
                u.sync_type == "semaphore"
                and u.id == pe_id
                and u.update_mode in ("sem-add-imm", "sem-sub-imm")
            ):
                assert u.update_value == len(events), (
                    f"unexpected bulk sem adjust {u.update_mode} "
                    f"{u.update_value} != {len(events)}"
                )
                u.update_value = len(kept)
    keep_set = set(kept)
    for i, inst in enumerate(events, 1):
        if i in keep_set:
            continue
        si = inst.sync_info
        new_upd = [
            u
            for u in si.on_update
            if not (u.sync_type == "semaphore" and u.id == pe_id)
        ]
        inst.sync_info = mybir.SyncInfo(
            on_wait=list(si.on_wait), on_update=new_upd
        )


def hoist_pe_waits(nc):
    """Move sem waits off InstLdweights/InstMatmult onto standalone PE
    EventSemaphore instructions inserted just before them. A wait-bearing
    engine instruction forces the PE NX out of hardware decode (~71ns SW
    decode + sem path vs 2.2ns); a seq-only wait keeps the engine stream
    HW-decoded. Ordering is identical: same engine, same program position.
    """
    fn = nc.m.functions[0]
    for bb in fn.blocks:
        # collect (index, inst) for wait-bearing PE engine instructions
        targets = []
        for i, inst in enumerate(bb.instructions):
            if isinstance(inst, (mybir.InstLdweights, mybir.InstMatmult)):
                si = inst.sync_info
                if si is not None and si.on_wait:
                    targets.append((i, inst))
        for i, inst in reversed(targets):
            si = inst.sync_info
            ev = mybir.InstEventSemaphore(
                name=nc.get_next_instruction_name(), ins=[], outs=[]
            )
            ev.engine = inst.engine
            ev.sync_info = mybir.SyncInfo(
                on_wait=list(si.on_wait), on_update=[]
            )
            inst.sync_info = mybir.SyncInfo(
                on_wait=[], on_update=list(si.on_update)
            )
            nc.register_instruction(ev)
            bb.instructions.insert(i, ev)


def build_nc(reps_loop=False, max_reps=512, mm=None, strip=True, ver=4, parts="full", store_q="split", hoist=False, kcontig=True):
    mmd = MM_DTYPES[mm or MM][0]
    """Build the per-core Bass program. With reps_loop, the whole body runs
    inside a For_i whose trip count is read from an int32 input "reps"."""
    nc = bacc.Bacc(None, target_bir_lowering=False)
    with tile.TileContext(nc) as tc:
        xt = nc.dram_tensor("xt", [NCT, P, KD, CT], mmd, kind="ExternalInput")
        gw = nc.dram_tensor("gw", [NH4, P, KD, 512], mmd, kind="ExternalInput")
        uw = nc.dram_tensor("uw", [NH4, P, KD, 512], mmd, kind="ExternalInput")
        dw = nc.dram_tensor("dw", [2, 2, P, KH // 2, 512], mmd, kind="ExternalInput")
        pv = nc.dram_tensor("pv", [P, C // P], F32, kind="ExternalInput")
        y = nc.dram_tensor("y", [NCT, 2, P, CT // P, 512], F32, kind="ExternalOutput")
        if ver == 4:
            def emit(warm):
                emit_expert_ffn_v4(tc, xt, gw, uw, dw, pv, y, MMD=mmd, warm=warm, parts=parts, store_q=store_q, kcontig=kcontig)
        elif ver == 3:
            def emit(warm):
                emit_expert_ffn_v3(tc, xt, gw, uw, dw, pv, y, MMD=mmd, warm=warm)
        elif ver == 2:
            def emit(warm):
                emit_expert_ffn_v2(tc, xt, gw, uw, dw, pv, y, MMD=mmd, warm=warm)
        else:
            def emit(warm):
                emit_expert_ffn(tc, xt, gw, uw, dw, pv, y, MMD=mmd)
        if reps_loop:
            reps = nc.dram_tensor("reps", [1, 1], mybir.dt.int32, kind="ExternalInput")
            with tc.tile_pool(name="rpool", bufs=1) as rpool, \
                 tc.tile_pool(name="spool", bufs=1) as spool:
                r_sb = rpool.tile([1, 1], mybir.dt.int32)
                nc.sync.dma_start(r_sb[:, :], reps[:, :])
                rv = nc.values_load(
                    r_sb[0:1, 0:1],
                    min_val=0,
                    max_val=max_reps,
                    skip_runtime_bounds_check=True,
                )
                warm = None
                if ver >= 2:
                    warm = spool.tile([P, 512], mmd, name="warm")
                    nc.vector.memset(warm[:, :], 0.0)
                with tc.For_i(0, rv, 1):
                    emit(warm)
        else:
            with tc.tile_pool(name="spool", bufs=1) as spool:
                warm = None
                if ver >= 2:
                    warm = spool.tile([P, 512], mmd, name="warm")
                    nc.vector.memset(warm[:, :], 0.0)
                emit(warm)
    nc.compile()
    # strip AFTER compile: the compile passes (move_matmul_waits_to_ldweights,
    # generate_event_semaphores, loop lowering) re-derive sem totals, so
    # rewriting before them leaves stale counts behind
    if strip:
        strip_pe_incs(nc)
    if hoist:
        hoist_pe_waits(nc)
    return nc


def pack_inputs(x_pad, gate_w_e, up_w_e, down_w_e, p_pad, mm=None):
    npdt = MM_DTYPES[mm or MM][1]
    """Pack one expert's inputs into the SBUF-tile-order DRAM layouts.
    Matmul operands are cast to bf16 (cast first: halves the transpose
    bytes)."""
    # xt [NCT, 128, KD, 512]: [ct, p, kt, tok] = x_pad[ct*512+tok, kt*128+p]
    xt = np.ascontiguousarray(
        x_pad.astype(npdt).reshape(NCT, CT, KD, P).transpose(0, 3, 2, 1)
    )
    # gw/uw [NH4, 128, KD, 512]: [b, p, kt, h] = w[b*512+h, kt*128+p]
    gw = np.ascontiguousarray(
        gate_w_e.astype(npdt).reshape(NH4, 512, KD, P).transpose(0, 3, 2, 1)
    )
    uw = np.ascontiguousarray(
        up_w_e.astype(npdt).reshape(NH4, 512, KD, P).transpose(0, 3, 2, 1)
    )
    # dw [2, 2, 128, KH//2, 512]: [dc, hf, p, kb, d] = down[dc*512+d, hf*1024+kb*128+p]
    dw = np.ascontiguousarray(
        down_w_e.astype(npdt).reshape(2, 512, 2, KH // 2, P).transpose(0, 2, 4, 3, 1)
    )
    pv = np.ascontiguousarray(p_pad.reshape(C // P, P).T)
    return {"xt": xt, "gw": gw, "uw": uw, "dw": dw, "pv": pv}


def unpack_y(y_pack):
    """y_pack [NCT, 2, 128, 4, 512] -> y [C, D]."""
    return np.ascontiguousarray(
        y_pack.transpose(0, 3, 2, 1, 4).reshape(C, D)
    )


def route_and_dispatch(x, router_w):
    """Host router + top-2 dispatch (matches softmax/top_k/renorm of the
    reference exactly)."""
    logits = x @ router_w.T                      # [T, E]
    t_ar = np.arange(T)
    i1 = np.argmax(logits, axis=1)
    l1 = logits[t_ar, i1]
    lm = logits.copy()
    lm[t_ar, i1] = -np.inf
    i2 = np.argmax(lm, axis=1)
    l2 = lm[t_ar, i2]
    e2 = np.exp(l2 - l1)
    p1 = 1.0 / (1.0 + e2)
    p2 = e2 / (1.0 + e2)

    ee = np.concatenate([i1, i2])                # [2T] expert of each pair
    tt = np.concatenate([t_ar, t_ar])            # [2T] token of each pair
    pp = np.concatenate([p1, p2]).astype(np.float32)
    counts = np.bincount(ee, minlength=E)
    starts = np.zeros(E, np.int64)
    starts[1:] = np.cumsum(counts)[:-1]
    order = np.argsort(ee, kind="stable")
    pos = np.empty(2 * T, np.int64)
    pos[order] = np.arange(2 * T) - starts[ee[order]]
    return ee, tt, pp, pos, counts, starts, order


def kernel(**inputs):
    x = np.ascontiguousarray(
        np.asarray(inputs["hidden_states"], np.float32).reshape(T, D)
    )
    router_w = np.asarray(inputs["router_w"], np.float32)
    gate_w = np.asarray(inputs["gate_w"], np.float32)
    up_w = np.asarray(inputs["up_w"], np.float32)
    down_w = np.asarray(inputs["down_w"], np.float32)

    ee, tt, pp, pos, counts, starts, order = route_and_dispatch(x, router_w)

    in_maps = []
    for e in range(E):
        n_e = min(int(counts[e]), C)
        sel = order[starts[e]:starts[e] + n_e]   # pairs dispatched to core e
        xp = np.zeros((C, D), np.float32)
        xp[:n_e] = x[tt[sel]]
        pvec = np.zeros(C, np.float32)
        pvec[:n_e] = pp[sel]
        in_maps.append(pack_inputs(xp, gate_w[e], up_w[e], down_w[e], pvec))

    nc = build_nc()
    res = run_bass_kernel_spmd(nc, in_maps, core_ids=list(range(E)))
    ys = np.stack(
        [unpack_y(res.results[e]["y"]) for e in range(E)]
    ).reshape(E * C, D)

    ok = pos < C
    contrib = np.zeros((2 * T, D), np.float32)
    g = ee * C + pos
    contrib[ok] = ys[g[ok]]
    # capacity-overflow fallback: exact fp32 host compute for the few pairs
    # beyond capacity (~0.8% of pairs for the seed-0 routing), batched per
    # expert
    if not ok.all():
        bad = np.nonzero(~ok)[0]
        for e in np.unique(ee[bad]):
            js = bad[ee[bad] == e]
            xb = x[tt[js]]
            gb = xb @ gate_w[e].T
            ub = xb @ up_w[e].T
            hb = (gb / (1.0 + np.exp(-gb))) * ub
            contrib[js] = (hb @ down_w[e].T) * pp[js, None]
    out = contrib[:T] + contrib[T:]
    return out.reshape(B, S, D).astype(np.float32)

